# revision 1
# baseline (speedup 1.0000x reference)
"""3-layer GAT (PyG GATConv semantics) on 8 Trainium2 NeuronCores.

Strategy (dst-sharded, gather-based):
- Nodes are assigned to 160 blocks of <=128 dst nodes, degree-balanced; 20 blocks per core.
- Per layer: each core computes its shard's dense projection h_aug = hprev @ [W | W@a_src | W@a_dst]
  (f32 matmuls), writes an fp16 row table [slots, 264], AllGathers the table (Local output;
  Shared-output collectives clobber row 0 on this runtime).
- Edge phase per block: one [128,1] indirect-DMA gather per 128-edge tile fetches h_aug rows
  by edge src (fp16, 528B; this runtime's indirect DMA honors exactly one index per partition
  and only 2-D output APs). Per-edge alpha_dst needs no gather: the S selector tile is
  PE-transposed and matmul'd against the resident per-block alpha_dst column. Then
  e = leaky(alpha_src + alpha_dst), ex = exp(e) (f32 math), messages m = ex * h (fp16),
  aggregation + softmax denominators via one PE matmul per 128-edge tile:
  lhsT = S (0/1 edge->dstslot matrix built by iota-compare), rhs = [m | ex] -> PSUM [128, 260].
- Softmax applied after aggregation: out = psum[:, :256] / denom (per head), + bias, ELU.
- Layer 3 (heads=1, C=1) same scheme with scalar tables.

The walrus in this toolchain accepts only ONE sync wait per instruction; BassOneWait
splits Tile-generated multi-waits into single-wait EventSemaphore ops at serialization.
"""
import numpy as np
from contextlib import ExitStack
import heapq

import orjson
import concourse.bass as bass
import concourse.tile as tile
from concourse import mybir
from concourse.bass_utils import run_bass_kernel_spmd

# problem constants (fixed by the harness's setup_inputs)
N_NODES = 20000
N_EDGES = 320000
IN_DIM = 128
HID = 64
HEADS = 4
HC = HEADS * HID          # 256
AUG = HC + 2 * HEADS      # 264 = h | alpha_src | alpha_dst
NEG = 0.2
NCORES = 8
P = 128
NBLK = 20                 # dst blocks per core
SLOTS = NBLK * P          # 2560 slots per core
TOT_SLOTS = SLOTS * NCORES

F32 = mybir.dt.float32
F32R = mybir.dt.float32r
F16 = mybir.dt.float16
I32 = mybir.dt.int32


def _split_multiwaits(bir: bytes) -> bytes:
    """Walrus here allows only 1 sync wait per instruction -> hoist extras onto
    same-engine EventSemaphore waits (dedup repeated ge-waits per engine; sems
    are monotonic within the block, so a repeated >= wait is a no-op)."""
    j = orjson.loads(bir)
    ctr = 0
    for fn in j["functions"]:
        for blk in fn["blocks"]:
            out_l = []
            last_wait = {}   # engine -> set of (id, value) already waited at this point
            for ins in blk["instructions"]:
                eng = ins.get("engine")
                si = ins.get("sync_info")
                ow = (si or {}).get("on_wait") or []
                ou = (si or {}).get("on_update") or []
                keep = 1
                if len(ow) > keep:
                    seen = last_wait.setdefault(eng, set())
                    for w in ow[:len(ow) - keep]:
                        key = (w.get("id"), w.get("wait_mode"), w.get("wait_value"))
                        if w.get("wait_mode") == "sem-ge-imm":
                            if key in seen:
                                continue
                            seen.add(key)
                        ctr += 1
                        out_l.append({
                            "engine": eng, "ins": [], "outs": [],
                            "name": f"mwsplit-{ctr}", "opcode": "EventSemaphore",
                            "sync_info": {"on_update": [], "on_wait": [w]},
                        })
                    si["on_wait"] = ow[len(ow) - keep:]
                out_l.append(ins)
            blk["instructions"] = out_l
    return orjson.dumps(j)


class BassOneWait(bass.Bass):
    def to_json_bytes(self):
        return _split_multiwaits(super().to_json_bytes())


# ---------------------------------------------------------------- host prep

def _preprocess(edge_index):
    """Assign nodes to degree-balanced blocks; build per-core edge tile arrays."""
    src = np.asarray(edge_index[0], dtype=np.int64)
    dst = np.asarray(edge_index[1], dtype=np.int64)
    loops = np.arange(N_NODES, dtype=np.int64)
    src = np.concatenate([src, loops])
    dst = np.concatenate([dst, loops])
    deg = np.bincount(dst, minlength=N_NODES).astype(np.int64)

    NB_TOT = NCORES * NBLK
    # greedy LPT: highest degree first onto least-loaded block with space
    order = np.argsort(-deg, kind="stable")
    blk_of = np.empty(N_NODES, np.int32)
    slot_of = np.empty(N_NODES, np.int32)
    heap = [(0, 0, b) for b in range(NB_TOT)]
    heapq.heapify(heap)
    cnt = np.zeros(NB_TOT, np.int32)
    load = np.zeros(NB_TOT, np.int64)
    for n in order:
        while True:
            l, _, b = heapq.heappop(heap)
            if cnt[b] < P:
                break
        blk_of[n] = b
        slot_of[n] = cnt[b]
        cnt[b] += 1
        load[b] += deg[n]
        if cnt[b] < P:
            heapq.heappush(heap, (load[b], cnt[b], b))

    T = int(np.ceil(load.max() / P))  # edge tiles per block (same for all)
    gslot = blk_of.astype(np.int64) * P + slot_of        # global table row of node
    node_of_slot = np.full(NB_TOT * P, -1, np.int64)
    node_of_slot[gslot] = np.arange(N_NODES)

    # bucket edges by dst block
    eb = blk_of[dst]
    order_e = np.argsort(eb, kind="stable")
    src_s = src[order_e]
    dst_s = dst[order_e]
    eb_s = eb[order_e]
    starts = np.searchsorted(eb_s, np.arange(NB_TOT + 1))

    NT = NBLK * T
    srcg = np.zeros((NCORES, P, NT), np.int32)       # global table row of edge src
    dstl = np.zeros((NCORES, P, NT), np.int32)       # core-local slot of edge dst
    dblk = np.full((NCORES, P, NT), -1.0, np.float16)  # block-local dst slot (-1 pad)
    for b in range(NB_TOT):
        c, lb = divmod(b, NBLK)
        e0, e1 = starts[b], starts[b + 1]
        k = e1 - e0
        col = np.zeros(T * P, np.int64)
        col[:k] = gslot[src_s[e0:e1]]
        srcg[c, :, lb * T:(lb + 1) * T] = col.reshape(T, P).T
        col_d = np.zeros(T * P, np.int64)
        col_d[:k] = lb * P + slot_of[dst_s[e0:e1]]
        dstl[c, :, lb * T:(lb + 1) * T] = col_d.reshape(T, P).T
        col_b = np.full(T * P, -1.0, np.float32)
        col_b[:k] = slot_of[dst_s[e0:e1]]
        dblk[c, :, lb * T:(lb + 1) * T] = col_b.reshape(T, P).T.astype(np.float16)

    return T, gslot, node_of_slot, srcg, dstl, dblk


def _aug_weights(W, a_src, a_dst, heads, hid):
    """[W | ws | wd] with ws[:,h] = W[:, h*hid:(h+1)*hid] @ a_src[h]."""
    cin = W.shape[0]
    ws = np.zeros((cin, heads), np.float32)
    wd = np.zeros((cin, heads), np.float32)
    for h in range(heads):
        blk = W[:, h * hid:(h + 1) * hid]
        ws[:, h] = blk @ a_src[h]
        wd[:, h] = blk @ a_dst[h]
    return np.concatenate([W, ws, wd], axis=1).astype(np.float32)


# ---------------------------------------------------------------- device kernel

def _build(T):
    NT = NBLK * T
    nc = BassOneWait()
    dp = nc.declare_dram_parameter
    x_in = dp("x_in", [SLOTS, IN_DIM], F32, isOutput=False)
    srcg_in = dp("srcg_in", [P, NT], I32, isOutput=False)
    dstl_in = dp("dstl_in", [P, NT], I32, isOutput=False)
    dblk_in = dp("dblk_in", [P, NT], F16, isOutput=False)
    wa1_in = dp("wa1_in", [IN_DIM, AUG], F32, isOutput=False)
    wa2_in = dp("wa2_in", [HC, AUG], F32, isOutput=False)
    w3_in = dp("w3_in", [1, HC], F32, isOutput=False)
    c3_in = dp("c3_in", [1, 4], F32, isOutput=False)   # a_src3, a_dst3, b3, 0
    b1_in = dp("b1_in", [1, HC], F32, isOutput=False)
    b2_in = dp("b2_in", [1, HC], F32, isOutput=False)
    iota_in = dp("iota_in", [1, P], F16, isOutput=False)
    ident_in = dp("ident_in", [P, P], F32, isOutput=False)
    out_p = dp("out_p", [P, NBLK], F32, isOutput=True)

    # internal DRAM
    tab_sh = [nc.dram_tensor(f"tab_sh{l}", [SLOTS, AUG], F16) for l in (1, 2)]
    tab_full = [nc.dram_tensor(f"tab_full{l}", [TOT_SLOTS, AUG], F16) for l in (1, 2)]
    adl_dram = [nc.dram_tensor(f"adl{l}", [SLOTS, HEADS], F32) for l in (1, 2)]
    h3_sh = nc.dram_tensor("h3_sh", [SLOTS, 1], F32)
    tab3 = nc.dram_tensor("tab3", [TOT_SLOTS, 1], F32)

    groups = [list(range(NCORES))]

    with tile.TileContext(nc) as tc, ExitStack() as ctx:
        consts = ctx.enter_context(tc.tile_pool(name="consts", bufs=1))
        meta = ctx.enter_context(tc.tile_pool(name="meta", bufs=1))
        state = ctx.enter_context(tc.tile_pool(name="state", bufs=1))
        work = ctx.enter_context(tc.tile_pool(name="work", bufs=2))
        gpool = ctx.enter_context(tc.tile_pool(name="gpool", bufs=4))
        small = ctx.enter_context(tc.tile_pool(name="small", bufs=4))
        psd = ctx.enter_context(tc.tile_pool(name="psd", bufs=1, space="PSUM"))
        pse = ctx.enter_context(tc.tile_pool(name="pse", bufs=2, space="PSUM"))
        pst = ctx.enter_context(tc.tile_pool(name="pst", bufs=2, space="PSUM"))
        psa = ctx.enter_context(tc.tile_pool(name="psa", bufs=2, space="PSUM"))

        # ---- constants / metadata loads
        ident = consts.tile([P, P], F32)
        nc.sync.dma_start(out=ident, in_=ident_in[:])
        ident16 = consts.tile([P, P], F16)
        nc.vector.tensor_copy(out=ident16, in_=ident)
        wa1 = consts.tile([P, AUG], F32)
        nc.sync.dma_start(out=wa1, in_=wa1_in[:])
        wa2 = consts.tile([P, 2, AUG], F32)
        nc.sync.dma_start(out=wa2, in_=wa2_in.rearrange("(j p) a -> p j a", p=P))
        def rep_load(name, src, n, dt):
            t = consts.tile([P, n], dt, tag=name)
            bc = bass.AP(tensor=src.tensor, offset=0, ap=[[0, P], [1, n]])
            nc.sync.dma_start(out=t, in_=bc)
            return t
        w3r = rep_load("w3r", w3_in[:], HC, F32)
        c3 = rep_load("c3", c3_in[:], 4, F32)
        b1r = rep_load("b1r", b1_in[:], HC, F32)
        b2r = rep_load("b2r", b2_in[:], HC, F32)
        iot = rep_load("iot", iota_in[:], P, F16)

        srcg = meta.tile([P, NT], I32)
        nc.sync.dma_start(out=srcg, in_=srcg_in[:])
        dstl = meta.tile([P, NT], I32)
        nc.sync.dma_start(out=dstl, in_=dstl_in[:])
        dblk = meta.tile([P, NT], F16)
        nc.sync.dma_start(out=dblk, in_=dblk_in[:])

        xin = state.tile([P, NBLK, IN_DIM], F32)
        nc.sync.dma_start(out=xin, in_=x_in.rearrange("(b p) d -> p b d", p=P))

        hprev = state.tile([P, NBLK, HC], F32)   # layer-1 output
        hprev2 = state.tile([P, NBLK, HC], F32)  # layer-2 output
        hT = state.tile([P, 2 * NBLK, P], F32)   # transposed dense input

        def bcast_row(t, shape):
            # t is [P, n] partition-replicated; broadcast middle dims (stride 0)
            ap = [list(t.ap[0])]
            for s in shape[1:-1]:
                ap.append([0, s])
            ap.append([t.ap[-1][0], shape[-1]])
            return bass.AP(tensor=t.tensor, offset=t.offset, ap=ap)

        adl_sbs = {}
        def dense_layer(lidx, cin_tiles):
            """h_aug per block -> tab_sh[lidx], adl_dram[lidx]."""
            adl_sb = state.tile([P, NBLK, HEADS], F16, tag=f"adl_sb{lidx}")
            adl_sbs[lidx] = adl_sb
            for b in range(NBLK):
                ps = psd.tile([P, AUG], F32, tag="dense")
                for j in range(cin_tiles):
                    lhsT = hT[:, cin_tiles * b + j, :]
                    rhs = wa1[:, :] if lidx == 0 else wa2[:, j, :]
                    nc.tensor.matmul(ps, lhsT, rhs,
                                     start=(j == 0), stop=(j == cin_tiles - 1))
                tabt = small.tile([P, AUG], F16, tag="tabt")
                nc.vector.tensor_copy(out=tabt, in_=ps)
                nc.sync.dma_start(
                    out=tab_sh[lidx].rearrange("(b p) a -> p b a", p=P)[:, b, :],
                    in_=tabt)
                nc.vector.tensor_copy(out=adl_sb[:, b, :], in_=ps[:, HC + HEADS:AUG])

        def transpose_into(src_view, dst_col):
            """PE-transpose [128,128] src_view into hT[:, dst_col, :]."""
            tp = pst.tile([P, P], F32, tag="tr")
            nc.tensor.transpose(out=tp, in_=src_view, identity=ident)
            nc.vector.tensor_copy(out=hT[:, dst_col, :], in_=tp)

        def edge_layer(lidx, hout, brow):
            """Gather + attention + aggregate for layer lidx (0 or 1)."""
            for b in range(NBLK):
                sl = slice(b * T, (b + 1) * T)
                hg = gpool.tile([P, T, AUG], F16, tag="hg")
                for t in range(T):
                    gt = b * T + t
                    nc.gpsimd.indirect_dma_start(
                        out=hg[:, t, :], out_offset=None, in_=tab_full[lidx][:],
                        in_offset=bass.IndirectOffsetOnAxis(ap=srcg[:, gt:gt+1], axis=0))
                # S first; then per-tile alpha_dst via PE: (S_t)^T @ adl_block
                S = work.tile([P, T, P], F16, tag="S")
                db_b = bass.AP(tensor=dblk.tensor, offset=dblk[:, sl].offset,
                               ap=[dblk.ap[0], [dblk.ap[1][0], T], [0, P]])
                nc.vector.tensor_tensor(out=S, in0=db_b,
                                        in1=bcast_row(iot, [P, T, P]),
                                        op=mybir.AluOpType.is_equal)
                adx = gpool.tile([P, T, HEADS], F32, tag="adx")
                adl_b = adl_sbs[lidx]
                for t in range(T):
                    stp = pst.tile([P, P], F16, tag="tr")
                    nc.tensor.transpose(out=stp, in_=S[:, t, :], identity=ident16)
                    stt = small.tile([P, P], F16, tag="stt")
                    nc.vector.tensor_copy(out=stt, in_=stp)
                    adp = psa.tile([P, HEADS], F32, tag="adp")
                    nc.tensor.matmul(adp, stt, adl_b[:, b, :], start=True, stop=True)
                    nc.vector.tensor_copy(out=adx[:, t, :], in_=adp)

                asum = small.tile([P, T, HEADS], F32, tag="asum")
                nc.vector.tensor_copy(out=asum, in_=hg[:, :, HC:HC + HEADS])
                nc.vector.tensor_tensor(out=asum, in0=asum, in1=adx,
                                        op=mybir.AluOpType.add)
                lk = small.tile([P, T, HEADS], F32, tag="lk")
                nc.vector.tensor_scalar_mul(lk, asum, NEG)
                nc.vector.tensor_tensor(out=lk, in0=lk, in1=asum,
                                        op=mybir.AluOpType.max)
                exf = small.tile([P, T, HEADS], F16, tag="exf")
                nc.scalar.activation(out=exf, in_=lk,
                                     func=mybir.ActivationFunctionType.Exp)

                m = work.tile([P, T, HC + HEADS], F16, tag="m")
                ex_b = bass.AP(tensor=exf.tensor, offset=exf.offset,
                               ap=[exf.ap[0], exf.ap[1], exf.ap[2], [0, HID]])
                nc.vector.tensor_tensor(
                    out=m[:, :, 0:HC].rearrange("p t (h c) -> p t h c", h=HEADS),
                    in0=hg[:, :, 0:HC].rearrange("p t (h c) -> p t h c", h=HEADS),
                    in1=ex_b, op=mybir.AluOpType.mult)
                nc.vector.tensor_copy(out=m[:, :, HC:HC + HEADS], in_=exf)

                ps = pse.tile([P, HC + HEADS], F32, tag="agg")
                for t in range(T):
                    nc.tensor.matmul(ps, S[:, t, :], m[:, t, :],
                                     start=(t == 0), stop=(t == T - 1))

                den = small.tile([P, HEADS], F32, tag="den")
                nc.vector.tensor_scalar_max(den, ps[:, HC:HC + HEADS], 1e-30)
                rec = small.tile([P, HEADS], F32, tag="rec")
                nc.vector.reciprocal(out=rec, in_=den)
                rec_b = bass.AP(tensor=rec.tensor, offset=rec.offset,
                                ap=[rec.ap[0], rec.ap[1], [0, HID]])
                hn = small.tile([P, HC], F32, tag="hn")
                nc.vector.tensor_tensor(
                    out=hn.rearrange("p (h c) -> p h c", h=HEADS),
                    in0=ps[:, 0:HC].rearrange("p (h c) -> p h c", h=HEADS),
                    in1=rec_b, op=mybir.AluOpType.mult)
                # bias + ELU
                nc.vector.tensor_tensor(out=hn, in0=hn, in1=brow,
                                        op=mybir.AluOpType.add)
                emin = small.tile([P, HC], F32, tag="emin")
                nc.vector.tensor_scalar_min(emin, hn, 0.0)
                eex = small.tile([P, HC], F32, tag="eex")
                nc.scalar.activation(out=eex, in_=emin,
                                     func=mybir.ActivationFunctionType.Exp)
                nc.vector.tensor_scalar_max(hn, hn, 0.0)
                nc.vector.tensor_tensor(out=hn, in0=hn, in1=eex,
                                        op=mybir.AluOpType.add)
                nc.vector.tensor_scalar_add(hout[:, b, :], hn, -1.0)

        # ================= layer 1
        for b in range(NBLK):
            transpose_into(xin[:, b, :], b)
        dense_layer(0, 1)
        nc.gpsimd.collective_compute(
            "AllGather", mybir.AluOpType.bypass, replica_groups=groups,
            ins=[tab_sh[0][:]], outs=[tab_full[0][:]])
        edge_layer(0, hprev, b1r)

        # ================= layer 2
        for b in range(NBLK):
            transpose_into(hprev[:, b, 0:P], 2 * b)
            transpose_into(hprev[:, b, P:HC], 2 * b + 1)
        dense_layer(1, 2)
        nc.gpsimd.collective_compute(
            "AllGather", mybir.AluOpType.bypass, replica_groups=groups,
            ins=[tab_sh[1][:]], outs=[tab_full[1][:]])
        edge_layer(1, hprev2, b2r)

        # ================= layer 3 dense: h3 = hprev2 @ W3 + b3
        h3sb = state.tile([P, NBLK, 1], F32)
        for b in range(NBLK):
            tmp = small.tile([P, HC], F32, tag="l3tmp")
            nc.vector.tensor_tensor(out=tmp, in0=hprev2[:, b, :],
                                    in1=w3r,
                                    op=mybir.AluOpType.mult)
            nc.vector.tensor_reduce(out=h3sb[:, b, :], in_=tmp,
                                    axis=mybir.AxisListType.X,
                                    op=mybir.AluOpType.add)
        h316 = state.tile([P, NBLK, 1], F16)
        nc.vector.tensor_copy(out=h316, in_=h3sb)
        b3_b = bass.AP(tensor=c3.tensor, offset=c3[:, 2:3].offset,
                       ap=[list(c3.ap[0]), [0, NBLK], [0, 1]])
        nc.vector.tensor_tensor(out=h3sb, in0=h3sb, in1=b3_b,
                                op=mybir.AluOpType.add)
        nc.sync.dma_start(out=h3_sh.rearrange("(b p) o -> p b o", p=P), in_=h3sb)
        nc.gpsimd.collective_compute(
            "AllGather", mybir.AluOpType.bypass, replica_groups=groups,
            ins=[h3_sh[:]], outs=[tab3[:]])

        # ================= layer 3 edge phase
        outsb = state.tile([P, NBLK], F32)
        a3s_b = lambda sh: bass.AP(tensor=c3.tensor, offset=c3[:, 0:1].offset,
                                   ap=[list(c3.ap[0]), [0, sh[1]], [0, 1]])
        a3d_b = lambda sh: bass.AP(tensor=c3.tensor, offset=c3[:, 1:2].offset,
                                   ap=[list(c3.ap[0]), [0, sh[1]], [0, 1]])
        for b in range(NBLK):
            sl = slice(b * T, (b + 1) * T)
            g3 = gpool.tile([P, T, 1], F32, tag="g3")
            d3 = gpool.tile([P, T, 1], F32, tag="d3")
            for t in range(T):
                gt = b * T + t
                nc.gpsimd.indirect_dma_start(
                    out=g3[:, t, :], out_offset=None, in_=tab3[:],
                    in_offset=bass.IndirectOffsetOnAxis(ap=srcg[:, gt:gt+1], axis=0))
            S = work.tile([P, T, P], F16, tag="S")
            db_b = bass.AP(tensor=dblk.tensor, offset=dblk[:, sl].offset,
                           ap=[dblk.ap[0], [dblk.ap[1][0], T], [0, P]])
            nc.vector.tensor_tensor(out=S, in0=db_b,
                                    in1=bcast_row(iot, [P, T, P]),
                                    op=mybir.AluOpType.is_equal)
            for t in range(T):
                stp = pst.tile([P, P], F16, tag="tr")
                nc.tensor.transpose(out=stp, in_=S[:, t, :], identity=ident16)
                stt = small.tile([P, P], F16, tag="stt")
                nc.vector.tensor_copy(out=stt, in_=stp)
                adp = psa.tile([P, HEADS], F32, tag="adp")
                nc.tensor.matmul(adp[:, 0:1], stt, h316[:, b, :], start=True, stop=True)
                nc.vector.tensor_copy(out=d3[:, t, :], in_=adp[:, 0:1])
            e3 = small.tile([P, T, 1], F32, tag="e3")
            t3 = small.tile([P, T, 1], F32, tag="t3")
            nc.vector.tensor_tensor(out=e3, in0=g3, in1=a3s_b([P, T]),
                                    op=mybir.AluOpType.mult)
            nc.vector.tensor_tensor(out=t3, in0=d3, in1=a3d_b([P, T]),
                                    op=mybir.AluOpType.mult)
            nc.vector.tensor_tensor(out=e3, in0=e3, in1=t3, op=mybir.AluOpType.add)
            nc.vector.tensor_scalar_mul(t3, e3, NEG)
            nc.vector.tensor_tensor(out=e3, in0=e3, in1=t3, op=mybir.AluOpType.max)
            ex3 = small.tile([P, T, 1], F32, tag="ex3")
            nc.scalar.activation(out=ex3, in_=e3,
                                 func=mybir.ActivationFunctionType.Exp)
            m3 = small.tile([P, T, 2], F16, tag="m3")
            nc.vector.tensor_tensor(out=m3[:, :, 0:1], in0=ex3, in1=g3,
                                    op=mybir.AluOpType.mult)
            nc.vector.tensor_copy(out=m3[:, :, 1:2], in_=ex3)
            ps3f = pse.tile([P, HC + HEADS], F32, tag="agg")
            ps3 = ps3f[:, 0:2]
            for t in range(T):
                nc.tensor.matmul(ps3, S[:, t, :], m3[:, t, :],
                                 start=(t == 0), stop=(t == T - 1))
            den3 = small.tile([P, 1], F32, tag="den3")
            nc.vector.tensor_scalar_max(den3, ps3[:, 1:2], 1e-30)
            rec3 = small.tile([P, 1], F32, tag="rec3")
            nc.vector.reciprocal(out=rec3, in_=den3)
            nc.vector.tensor_tensor(out=outsb[:, b:b + 1], in0=ps3[:, 0:1],
                                    in1=rec3, op=mybir.AluOpType.mult)
        nc.sync.dma_start(out=out_p[:], in_=outsb)

    return nc


_CACHE = {}


def kernel(x, edge_index, W1, a_src1, a_dst1, b1, W2, a_src2, a_dst2, b2,
           W3, a_src3, a_dst3, b3):
    T, gslot, node_of_slot, srcg, dstl, dblk = _preprocess(np.asarray(edge_index))

    wa1 = _aug_weights(np.asarray(W1, np.float32), np.asarray(a_src1, np.float32),
                       np.asarray(a_dst1, np.float32), HEADS, HID)
    wa2 = _aug_weights(np.asarray(W2, np.float32), np.asarray(a_src2, np.float32),
                       np.asarray(a_dst2, np.float32), HEADS, HID)
    w3 = np.asarray(W3, np.float32).reshape(1, HC)
    c3 = np.array([[float(np.asarray(a_src3).reshape(-1)[0]),
                    float(np.asarray(a_dst3).reshape(-1)[0]),
                    float(np.asarray(b3).reshape(-1)[0]), 0.0]], np.float32)
    iota = np.arange(P, dtype=np.float16).reshape(1, P)
    b1r = np.asarray(b1, np.float32).reshape(1, HC)
    b2r = np.asarray(b2, np.float32).reshape(1, HC)

    x = np.asarray(x, np.float32)
    in_maps = []
    for c in range(NCORES):
        sl = slice(c * SLOTS, (c + 1) * SLOTS)
        nos = node_of_slot[sl]
        xs = np.zeros((SLOTS, IN_DIM), np.float32)
        valid = nos >= 0
        xs[valid] = x[nos[valid]]
        in_maps.append({
            "x_in": xs,
            "srcg_in": srcg[c], "dstl_in": dstl[c], "dblk_in": dblk[c],
            "wa1_in": wa1, "wa2_in": wa2, "w3_in": w3, "c3_in": c3,
            "b1_in": b1r, "b2_in": b2r, "iota_in": iota,
            "ident_in": np.eye(P, dtype=np.float32),
        })

    if T not in _CACHE:
        _CACHE[T] = _build(T)
    nc = _CACHE[T]
    res = run_bass_kernel_spmd(nc, in_maps, list(range(NCORES)))

    out = np.empty(N_NODES, np.float32)
    for c in range(NCORES):
        o = res.results[c]["out_p"]          # [P, NBLK]
        flat = o.T.reshape(-1)               # slot-major: b*P + p
        nos = node_of_slot[c * SLOTS:(c + 1) * SLOTS]
        valid = nos >= 0
        out[nos[valid]] = flat[valid]
    return out



# revision 12
# speedup vs baseline: 1.7029x; 1.7029x over previous
"""3-layer GAT (PyG GATConv semantics) on 8 Trainium2 NeuronCores — v2.

Strategy (dst-sharded, CSR-ELL, batched dma_gather):
- Nodes sorted by in-degree, grouped into 20 degree-bands of 1024; band g gives
  one 128-node block to each of the 8 cores, with a shared column count
  Tg[g] = max degree in the band. Edge layout per block is ELL: partition =
  dst slot, free column j = j-th incoming edge. Degree sorting bounds ELL
  padding at ~8%.
- Per layer: dense phase computes h_aug = h @ [W | W@a_src | W@a_dst] per block
  ([128, 264] PSUM); rows [h(256) | asrc(4)] are written fp16 into a 768B-stride
  table shard, alpha_dst stays resident in SBUF. One AllGather publishes the
  full 20480-row table.
- Edge phase per block: batched InstDMAGatherAnt (mlp GPSIMD library, int16
  indices, <=1024 idxs per instruction — ring limit) pulls h_aug rows of all
  edge sources straight into ELL position [dst_slot, j]. alpha_dst needs no
  gather (partition == dst slot: broadcast of the resident column), attention
  weights are exp(leaky(asrc+adst) - 4.16 + mask) (mask kills ELL padding),
  messages multiply in place, aggregation + softmax denominator are a free-axis
  halving-tree fold. No selector matmuls, no PE transposes in the edge phase.
- Layer 3 (heads=1, C=1): same scheme over a 256B-row scalar table; h3[dst] is
  the resident dense output, broadcast along the free axis.

The walrus in this toolchain accepts only ONE sync wait per instruction;
BassOneWait splits Tile-generated multi-waits into single-wait EventSemaphore
ops at serialization.
"""
import numpy as np
from contextlib import ExitStack

import orjson
import concourse.bass as bass
import concourse.tile as tile
from concourse import mybir, library_config
from concourse.library_overlay import lower_extended_insts
from concourse.bass_utils import run_bass_kernel_spmd

# problem constants (fixed by the harness's setup_inputs)
N_NODES = 20000
N_EDGES = 320000
IN_DIM = 128
HID = 64
HEADS = 4
HC = HEADS * HID          # 256
AUG = HC + 2 * HEADS      # 264 = h | asrc | adst
ROWE = 384                # table row stride in fp16 elems (768B, 256B-multiple)
ROW3 = 128                # layer-3 table row stride in fp16 elems (256B)
NEG = 0.2
NCORES = 8
P = 128
NBLK = 20                 # dst blocks per core (degree bands)
SLOTS = NBLK * P          # 2560 slots per core
TOT_SLOTS = SLOTS * NCORES
NPAD = TOT_SLOTS          # 20480 (480 pad slots)
GMAX = 1024               # max indices per dma_gather (SWDGE ring limit)
NQUEUES = 4               # SWDGE queues (round-robin gathers across Q7 rings)
MASKV = -30000.0          # additive mask for ELL pad columns
EXP_SHIFT = -4.158883083359672   # ln(1/64): guards fp16 fold overflow

F32 = mybir.dt.float32
F16 = mybir.dt.float16
I16 = mybir.dt.int16

AF = mybir.ActivationFunctionType
OP = mybir.AluOpType


def _split_multiwaits(bir: bytes) -> bytes:
    """Walrus here allows only 1 sync wait per instruction -> hoist extras onto
    same-engine EventSemaphore waits (dedup repeated ge-waits per engine; sems
    are monotonic within the block, so a repeated >= wait is a no-op)."""
    j = orjson.loads(bir)
    ctr = 0
    for fn in j["functions"]:
        for blk in fn["blocks"]:
            out_l = []
            last_wait = {}   # engine -> set of (id, value) already waited at this point
            for ins in blk["instructions"]:
                eng = ins.get("engine")
                si = ins.get("sync_info")
                ow = (si or {}).get("on_wait") or []
                keep = 1
                if len(ow) > keep:
                    seen = last_wait.setdefault(eng, set())
                    for w in ow[:len(ow) - keep]:
                        key = (w.get("id"), w.get("wait_mode"), w.get("wait_value"))
                        if w.get("wait_mode") == "sem-ge-imm":
                            if key in seen:
                                continue
                            seen.add(key)
                        ctr += 1
                        out_l.append({
                            "engine": eng, "ins": [], "outs": [],
                            "name": f"mwsplit-{ctr}", "opcode": "EventSemaphore",
                            "sync_info": {"on_update": [], "on_wait": [w]},
                        })
                    si["on_wait"] = ow[len(ow) - keep:]
                out_l.append(ins)
            blk["instructions"] = out_l
    return orjson.dumps(j)


class BassOneWait(bass.Bass):
    def to_json_bytes(self):
        return _split_multiwaits(super().to_json_bytes())


# ---------------------------------------------------------------- host prep

def _preprocess(edge_index):
    """Degree-sorted band assignment + ELL edge layout + gather index arrays."""
    src = np.asarray(edge_index[0], dtype=np.int64)
    dst = np.asarray(edge_index[1], dtype=np.int64)
    loops = np.arange(N_NODES, dtype=np.int64)
    src = np.concatenate([src, loops])
    dst = np.concatenate([dst, loops])

    deg = np.zeros(NPAD, np.int64)
    deg[:N_NODES] = np.bincount(dst, minlength=N_NODES)

    order = np.argsort(-deg, kind="stable")          # rank -> node
    rank = np.empty(NPAD, np.int64)
    rank[order] = np.arange(NPAD)

    Tg = tuple(max(int(deg[order[g * 1024]]), 1) for g in range(NBLK))
    goff = np.concatenate([[0], np.cumsum(Tg)])
    NTT = int(goff[-1])

    g_of = rank // 1024
    w = rank % 1024
    c_of = w // P
    p_of = w % P
    gslot = c_of * SLOTS + g_of * P + p_of           # node -> global table row

    # per-edge placement: j = index among d's incoming edges
    eord = np.argsort(dst, kind="stable")
    dsts = dst[eord]
    srcs = src[eord]
    starts = np.searchsorted(dsts, np.arange(N_NODES + 1))
    j = np.arange(len(dsts)) - starts[dsts]

    ec = c_of[dsts]
    ep = p_of[dsts]
    ecol = goff[g_of[dsts]] + j

    idx_flat = np.zeros((NCORES, NTT * P), np.int16)
    idx_flat[ec, ecol * P + ep] = gslot[srcs].astype(np.int16)
    mask = np.full((NCORES, P, NTT), MASKV, np.float16)
    mask[ec, ep, ecol] = 0.0

    idx_w = np.empty((NCORES, P, NTT * 8), np.int16)
    for c in range(NCORES):
        w16 = idx_flat[c].reshape(NTT * 8, 16).T     # [16, NTT*8]
        idx_w[c] = np.tile(w16, (8, 1))

    return Tg, NTT, order, idx_w, mask


def _aug_weights(W, a_src, a_dst, heads, hid):
    """[W | ws | wd] with ws[:,h] = W[:, h*hid:(h+1)*hid] @ a_src[h]."""
    cin = W.shape[0]
    ws = np.zeros((cin, heads), np.float32)
    wd = np.zeros((cin, heads), np.float32)
    for h in range(heads):
        blk = W[:, h * hid:(h + 1) * hid]
        ws[:, h] = blk @ a_src[h]
        wd[:, h] = blk @ a_dst[h]
    return np.concatenate([W, ws, wd], axis=1).astype(np.float32)


# ---------------------------------------------------------------- device kernel

def _build(Tg):
    NTT = sum(Tg)
    TMAX = max(Tg)
    goff = [0]
    for t in Tg:
        goff.append(goff[-1] + t)

    nc = BassOneWait(num_swdge_queues=NQUEUES)
    dp = nc.declare_dram_parameter
    x_in = dp("x_in", [SLOTS, IN_DIM], F32, isOutput=False)
    idx_in = dp("idx_in", [P, NTT * 8], I16, isOutput=False)
    mask_in = dp("mask_in", [P, NTT], F16, isOutput=False)
    wa1_in = dp("wa1_in", [IN_DIM, AUG], F32, isOutput=False)
    wa2_in = dp("wa2_in", [HC, AUG], F32, isOutput=False)
    w3_in = dp("w3_in", [1, HC], F32, isOutput=False)
    c3_in = dp("c3_in", [1, 4], F32, isOutput=False)   # a_src3, a_dst3, b3, 0
    b1_in = dp("b1_in", [1, HC], F32, isOutput=False)
    b2_in = dp("b2_in", [1, HC], F32, isOutput=False)
    ident_in = dp("ident_in", [P, P], F32, isOutput=False)
    out_p = dp("out_p", [P, NBLK], F32, isOutput=True)

    # internal DRAM
    tab_sh = [nc.dram_tensor(f"tab_sh{l}", [SLOTS, ROWE], F16) for l in (1, 2)]
    tab_full = [nc.dram_tensor(f"tab_full{l}", [TOT_SLOTS, ROWE], F16) for l in (1, 2)]
    tab3_sh = nc.dram_tensor("tab3_sh", [SLOTS, ROW3], F16)
    tab3_full = nc.dram_tensor("tab3_full", [TOT_SLOTS, ROW3], F16)

    groups = [list(range(NCORES))]

    with tile.TileContext(nc) as tc, ExitStack() as ctx:
        consts = ctx.enter_context(tc.tile_pool(name="consts", bufs=1))
        meta = ctx.enter_context(tc.tile_pool(name="meta", bufs=1))
        state = ctx.enter_context(tc.tile_pool(name="state", bufs=1))
        gpool = ctx.enter_context(tc.tile_pool(name="gpool", bufs=2))
        g3pool = ctx.enter_context(tc.tile_pool(name="g3pool", bufs=2))
        sm = ctx.enter_context(tc.tile_pool(name="sm", bufs=3))
        psd = ctx.enter_context(tc.tile_pool(name="psd", bufs=2, space="PSUM"))
        pst = ctx.enter_context(tc.tile_pool(name="pst", bufs=2, space="PSUM"))

        nc.gpsimd.load_library(library_config.mlp)

        # ---- constants / metadata
        ident = consts.tile([P, P], F32)
        nc.sync.dma_start(out=ident, in_=ident_in[:])
        wa1 = consts.tile([P, AUG], F32)
        nc.sync.dma_start(out=wa1, in_=wa1_in[:])
        wa2 = consts.tile([P, 2, AUG], F32)
        nc.sync.dma_start(out=wa2, in_=wa2_in.rearrange("(j p) a -> p j a", p=P))

        def rep_load(name, srct, n, dt):
            t = consts.tile([P, n], dt, tag=name)
            bc = bass.AP(tensor=srct.tensor, offset=0, ap=[[0, P], [1, n]])
            nc.sync.dma_start(out=t, in_=bc)
            return t
        w3r = rep_load("w3r", w3_in[:], HC, F32)
        c3 = rep_load("c3", c3_in[:], 4, F32)
        b1r = rep_load("b1r", b1_in[:], HC, F32)
        b2r = rep_load("b2r", b2_in[:], HC, F32)

        eshift = consts.tile([P, 1], F32, tag="eshift")
        nc.vector.memset(eshift[:], EXP_SHIFT)

        idx = meta.tile([P, NTT * 8], I16)
        nc.sync.dma_start(out=idx, in_=idx_in[:])
        msk = meta.tile([P, NTT], F16)
        nc.sync.dma_start(out=msk, in_=mask_in[:])

        xin = state.tile([P, NBLK, IN_DIM], F32)
        nc.sync.dma_start(out=xin, in_=x_in.rearrange("(b p) d -> p b d", p=P))

        hprev = state.tile([P, NBLK, HC], F32)
        hprev2 = state.tile([P, NBLK, HC], F32)
        hT = state.tile([P, 2 * NBLK, P], F32)
        adl = state.tile([P, NBLK, HEADS], F32)      # resident alpha_dst (per layer)
        h3sb = state.tile([P, NBLK, 1], F32)
        h316 = state.tile([P, NBLK, 1], F16)
        hd3 = state.tile([P, NBLK, 1], F32)          # a_dst3 * h3 per slot
        outsb = state.tile([P, NBLK], F32)

        def ap_of(t_slice, ap):
            return bass.AP(tensor=t_slice.tensor, offset=t_slice.offset, ap=ap)

        def transpose_into(src_view, dst_col):
            tp = pst.tile([P, P], F32, tag="tr")
            nc.tensor.transpose(out=tp, in_=src_view, identity=ident)
            nc.vector.tensor_copy(out=hT[:, dst_col, :], in_=tp)

        def dense_layer(lidx, cin_tiles):
            for g in range(NBLK):
                if lidx == 0:
                    transpose_into(xin[:, g, :], g)
                else:
                    transpose_into(hprev[:, g, 0:P], 2 * g)
                    transpose_into(hprev[:, g, P:HC], 2 * g + 1)
                ps = psd.tile([P, AUG], F32, tag="dense")
                for jj in range(cin_tiles):
                    lhsT = hT[:, cin_tiles * g + jj, :]
                    rhs = wa1[:, :] if lidx == 0 else wa2[:, jj, :]
                    nc.tensor.matmul(ps, lhsT, rhs,
                                     start=(jj == 0), stop=(jj == cin_tiles - 1))
                tabt = sm.tile([P, HC + HEADS], F16, tag="tabt")
                nc.vector.tensor_copy(out=tabt, in_=ps[:, 0:HC + HEADS])
                nc.sync.dma_start(
                    out=tab_sh[lidx].rearrange("(g p) e -> p g e", p=P)[:, g, 0:HC + HEADS],
                    in_=tabt)
                nc.vector.tensor_copy(out=adl[:, g, :], in_=ps[:, HC + HEADS:AUG])

        nidx_regs = {}
        qctr = [0]

        def gathers(table, elem, out_tile, g):
            T = Tg[g]
            c0 = 0
            while c0 < T:
                ncols = min(GMAX // P, T - c0)
                if ncols not in nidx_regs:
                    nidx_regs[ncols] = nc.gpsimd.to_reg(P * ncols)
                col = goff[g] + c0
                nc.gpsimd.dma_gather(
                    out_ap=out_tile[:, c0:c0 + ncols, :],
                    in_ap=table[:],
                    idxs_ap=idx[:, 8 * col: 8 * (col + ncols)],
                    num_idxs=P * ncols, num_idxs_reg=nidx_regs[ncols],
                    elem_size=elem, queue_num=qctr[0])
                qctr[0] = (qctr[0] + 1) % NQUEUES
                c0 += ncols

        def edge_layer(lidx, hout, brow):
            for g in range(NBLK):
                T = Tg[g]
                hg = gpool.tile([P, TMAX, ROWE], F16, tag="hg")
                gathers(tab_full[lidx], ROWE, hg, g)
                # e = asrc[src] + adst[dst slot]  -> leaky -> +mask -> exp
                e = sm.tile([P, TMAX, HEADS], F32, tag="e")
                adl_b = ap_of(adl[:, g, :],
                              [list(adl.ap[0]), [0, T], [1, HEADS]])
                nc.vector.tensor_tensor(out=e[:, :T, :], in0=hg[:, :T, HC:HC + HEADS],
                                        in1=adl_b, op=OP.add)
                lk = sm.tile([P, TMAX, HEADS], F32, tag="lk")
                nc.vector.tensor_scalar_mul(lk[:, :T, :], e[:, :T, :], NEG)
                nc.vector.tensor_tensor(out=lk[:, :T, :], in0=lk[:, :T, :],
                                        in1=e[:, :T, :], op=OP.max)
                msk_b = ap_of(msk[:, goff[g]:goff[g] + T],
                              [list(msk.ap[0]), [msk.ap[1][0], T], [0, HEADS]])
                nc.vector.tensor_tensor(out=lk[:, :T, :], in0=lk[:, :T, :],
                                        in1=msk_b, op=OP.add)
                exf = sm.tile([P, TMAX, HEADS], F16, tag="exf")
                nc.scalar.activation(out=exf[:, :T, :], in_=lk[:, :T, :],
                                     func=AF.Exp, bias=eshift[:, :])
                # messages in place: hg[:, :, 0:256] *= exf (bcast over 64)
                exf_b = ap_of(exf[:, 0:T, :],
                              [list(exf.ap[0]), [HEADS, T], [1, HEADS], [0, HID]])
                hg4 = hg[:, 0:T, 0:HC].rearrange("p t (h c) -> p t h c", h=HEADS)
                nc.vector.tensor_tensor(out=hg4, in0=hg4, in1=exf_b, op=OP.mult)
                nc.vector.tensor_copy(out=hg[:, :T, HC:HC + HEADS], in_=exf[:, :T, :])
                # fold columns 0..T-1 into column 0 (num | den)
                n = T
                W = HC + HEADS
                while n > 1:
                    if n % 2 == 1:
                        nc.vector.tensor_tensor(
                            out=hg[:, 0:1, 0:W], in0=hg[:, 0:1, 0:W],
                            in1=hg[:, n - 1:n, 0:W], op=OP.add)
                        n -= 1
                    h = n // 2
                    nc.vector.tensor_tensor(
                        out=hg[:, 0:h, 0:W], in0=hg[:, 0:h, 0:W],
                        in1=hg[:, h:2 * h, 0:W], op=OP.add)
                    n = h
                den = sm.tile([P, HEADS], F32, tag="den")
                nc.vector.tensor_scalar_max(den, hg[:, 0, HC:HC + HEADS], 1e-30)
                rec = sm.tile([P, HEADS], F32, tag="rec")
                nc.vector.reciprocal(out=rec, in_=den)
                rec_b = ap_of(rec[:, :], [list(rec.ap[0]), [1, HEADS], [0, HID]])
                hn = sm.tile([P, HC], F32, tag="hn")
                nc.vector.tensor_tensor(
                    out=hn.rearrange("p (h c) -> p h c", h=HEADS),
                    in0=hg[:, 0, 0:HC].rearrange("p (h c) -> p h c", h=HEADS),
                    in1=rec_b, op=OP.mult)
                nc.vector.tensor_tensor(out=hn, in0=hn, in1=brow, op=OP.add)
                # ELU: max(x,0) + exp(min(x,0)) - 1  (exp on scalar engine)
                r = sm.tile([P, HC], F32, tag="r")
                nc.vector.tensor_scalar_min(r, hn, 0.0)
                eex = sm.tile([P, HC], F32, tag="eex")
                nc.scalar.activation(out=eex, in_=r, func=AF.Exp)
                nc.vector.tensor_scalar_max(hn, hn, 0.0)
                nc.vector.tensor_tensor(out=hn, in0=hn, in1=eex, op=OP.add)
                nc.vector.tensor_scalar_add(hout[:, g, :], hn, -1.0)

        # ================= layer 1
        dense_layer(0, 1)
        nc.gpsimd.collective_compute(
            "AllGather", OP.bypass, replica_groups=groups,
            ins=[tab_sh[0][:]], outs=[tab_full[0][:]])
        edge_layer(0, hprev, b1r)

        # ================= layer 2
        dense_layer(1, 2)
        nc.gpsimd.collective_compute(
            "AllGather", OP.bypass, replica_groups=groups,
            ins=[tab_sh[1][:]], outs=[tab_full[1][:]])
        edge_layer(1, hprev2, b2r)

        # ================= layer 3 dense: h3 = hprev2 @ W3 (bias added at end)
        for g in range(NBLK):
            tmp = sm.tile([P, HC], F32, tag="l3tmp")
            nc.vector.tensor_tensor(out=tmp, in0=hprev2[:, g, :], in1=w3r,
                                    op=OP.mult)
            nc.vector.tensor_reduce(out=h3sb[:, g, :], in_=tmp,
                                    axis=mybir.AxisListType.X, op=OP.add)
        nc.vector.tensor_copy(out=h316, in_=h3sb)
        ad3_col = ap_of(c3[:, 1:2], [list(c3.ap[0]), [0, NBLK], [0, 1]])
        nc.vector.tensor_tensor(out=hd3, in0=h3sb, in1=ad3_col, op=OP.mult)
        nc.sync.dma_start(
            out=tab3_sh.rearrange("(g p) e -> p g e", p=P)[:, :, 0:1], in_=h316)
        nc.gpsimd.collective_compute(
            "AllGather", OP.bypass, replica_groups=groups,
            ins=[tab3_sh[:]], outs=[tab3_full[:]])

        # ================= layer 3 edge phase
        for g in range(NBLK):
            T = Tg[g]
            g3 = g3pool.tile([P, TMAX, ROW3], F16, tag="g3")
            gathers(tab3_full, ROW3, g3, g)
            e3 = sm.tile([P, TMAX, 1], F32, tag="e3")
            as3_b = ap_of(c3[:, 0:1], [list(c3.ap[0]), [0, T], [0, 1]])
            nc.vector.tensor_tensor(out=e3[:, :T, :], in0=g3[:, :T, 0:1],
                                    in1=as3_b, op=OP.mult)
            hd3_b = ap_of(hd3[:, g, :], [list(hd3.ap[0]), [0, T], [1, 1]])
            nc.vector.tensor_tensor(out=e3[:, :T, :], in0=e3[:, :T, :],
                                    in1=hd3_b, op=OP.add)
            lk3 = sm.tile([P, TMAX, 1], F32, tag="lk3")
            nc.vector.tensor_scalar_mul(lk3[:, :T, :], e3[:, :T, :], NEG)
            nc.vector.tensor_tensor(out=lk3[:, :T, :], in0=lk3[:, :T, :],
                                    in1=e3[:, :T, :], op=OP.max)
            msk_b = ap_of(msk[:, goff[g]:goff[g] + T],
                          [list(msk.ap[0]), [msk.ap[1][0], T], [0, 1]])
            nc.vector.tensor_tensor(out=lk3[:, :T, :], in0=lk3[:, :T, :],
                                    in1=msk_b, op=OP.add)
            ex3 = sm.tile([P, TMAX, 1], F16, tag="ex3")
            nc.scalar.activation(out=ex3[:, :T, :], in_=lk3[:, :T, :],
                                 func=AF.Exp, bias=eshift[:, :])
            nc.vector.tensor_tensor(out=g3[:, :T, 0:1], in0=g3[:, :T, 0:1],
                                    in1=ex3[:, :T, :], op=OP.mult)
            nc.vector.tensor_copy(out=g3[:, :T, 1:2], in_=ex3[:, :T, :])
            n = T
            while n > 1:
                if n % 2 == 1:
                    nc.vector.tensor_tensor(
                        out=g3[:, 0:1, 0:2], in0=g3[:, 0:1, 0:2],
                        in1=g3[:, n - 1:n, 0:2], op=OP.add)
                    n -= 1
                h = n // 2
                nc.vector.tensor_tensor(
                    out=g3[:, 0:h, 0:2], in0=g3[:, 0:h, 0:2],
                    in1=g3[:, h:2 * h, 0:2], op=OP.add)
                n = h
            den3 = sm.tile([P, 1], F32, tag="den3")
            nc.vector.tensor_scalar_max(den3, g3[:, 0, 1:2], 1e-30)
            rec3 = sm.tile([P, 1], F32, tag="rec3")
            nc.vector.reciprocal(out=rec3, in_=den3)
            nc.vector.tensor_tensor(out=outsb[:, g:g + 1], in0=g3[:, 0, 0:1],
                                    in1=rec3, op=OP.mult)
        b3_b = ap_of(c3[:, 2:3], [list(c3.ap[0]), [0, NBLK]])
        nc.vector.tensor_tensor(out=outsb, in0=outsb, in1=b3_b, op=OP.add)
        nc.sync.dma_start(out=out_p[:], in_=outsb)

    lower_extended_insts(nc)
    return nc


_CACHE = {}


def kernel(x, edge_index, W1, a_src1, a_dst1, b1, W2, a_src2, a_dst2, b2,
           W3, a_src3, a_dst3, b3):
    Tg, NTT, order, idx_w, mask = _preprocess(np.asarray(edge_index))

    wa1 = _aug_weights(np.asarray(W1, np.float32), np.asarray(a_src1, np.float32),
                       np.asarray(a_dst1, np.float32), HEADS, HID)
    wa2 = _aug_weights(np.asarray(W2, np.float32), np.asarray(a_src2, np.float32),
                       np.asarray(a_dst2, np.float32), HEADS, HID)
    w3 = np.asarray(W3, np.float32).reshape(1, HC)
    c3 = np.array([[float(np.asarray(a_src3).reshape(-1)[0]),
                    float(np.asarray(a_dst3).reshape(-1)[0]),
                    float(np.asarray(b3).reshape(-1)[0]), 0.0]], np.float32)
    b1r = np.asarray(b1, np.float32).reshape(1, HC)
    b2r = np.asarray(b2, np.float32).reshape(1, HC)

    x = np.asarray(x, np.float32)
    in_maps = []
    for c in range(NCORES):
        # slot (g, p) of core c holds node order[g*1024 + c*128 + p]
        r = (np.arange(NBLK)[:, None] * 1024 + c * P + np.arange(P)[None, :])
        nodes = order[r.reshape(-1)]                 # [2560] in slot-major order
        xs = np.zeros((SLOTS, IN_DIM), np.float32)
        valid = nodes < N_NODES
        xs[valid] = x[nodes[valid]]
        in_maps.append({
            "x_in": xs,
            "idx_in": idx_w[c], "mask_in": mask[c],
            "wa1_in": wa1, "wa2_in": wa2, "w3_in": w3, "c3_in": c3,
            "b1_in": b1r, "b2_in": b2r,
            "ident_in": np.eye(P, dtype=np.float32),
        })

    if Tg not in _CACHE:
        _CACHE[Tg] = _build(Tg)
    nc = _CACHE[Tg]
    res = run_bass_kernel_spmd(nc, in_maps, list(range(NCORES)))

    out = np.empty(N_NODES, np.float32)
    for c in range(NCORES):
        o = np.asarray(res.results[c]["out_p"])      # [P, NBLK]
        r = (np.arange(NBLK)[:, None] * 1024 + c * P + np.arange(P)[None, :])
        nodes = order[r.reshape(-1)]
        vals = o.T.reshape(-1)                       # slot-major: g*P + p
        valid = nodes < N_NODES
        out[nodes[valid]] = vals[valid]
    return out


# revision 15
# speedup vs baseline: 1.7686x; 1.0386x over previous
"""3-layer GAT (PyG GATConv semantics) on 8 Trainium2 NeuronCores — v4.

Strategy (dst-sharded, CSR-ELL, batched dma_gather, chunked collectives):
- Nodes sorted by in-degree, grouped into 20 degree-bands of 1024; band g gives
  one 128-node block to each of the 8 cores with a shared column count
  Tg[g] = max degree in the band. Edge layout per block is ELL: partition =
  dst slot, free column j = j-th incoming edge (~8% padding).
- Table rows are numbered CHUNK-MAJOR (4 chunks of 5 bands) so each chunk's
  AllGather reads/writes contiguous rows; chunk AGs of layer l+1 fire while
  layer l's edge phase is still running (dense of l+1 is interleaved per
  chunk into l's edge phase).
- Dense: h_aug = h @ [W | ws | 0.2ws | wd | 0.2wd] per block ([128, 272] PSUM);
  cols 0:264 ([h | asrc | 0.2asrc]) go fp16 into the 768B-stride table row,
  cols 264:272 (adst, 0.2adst) stay SBUF-resident.
- Edge phase per block: batched InstDMAGatherAnt (mlp GPSIMD library, int16
  indices, <=1024 idxs/instruction, round-robin over 4 SWDGE queues) pulls rows
  into ELL position. leaky = max(asrc+adst, 0.2asrc+0.2adst) (prescaled, no
  tensor_scalar), + pad mask, exp on the scalar engine with a -ln(64) bias
  (fp16 fold-overflow guard; cancels in the softmax ratio). Messages multiply
  in place; aggregation + denominator = free-axis halving-tree fold. Stages are
  software-pipelined across blocks (A(g) issued before B(g-1)) and the
  normalize+bias+ELU tail is batched per 5-block chunk.
- Layer 3 (heads=1, C=1): same scheme over a 256B-row scalar table; h3[dst]
  comes from the resident dense output.

The walrus in this toolchain accepts only ONE sync wait per instruction;
BassOneWait splits Tile-generated multi-waits into single-wait EventSemaphore
ops at serialization.
"""
import numpy as np
from contextlib import ExitStack

import orjson
import concourse.bass as bass
import concourse.tile as tile
from concourse import mybir, library_config
from concourse.library_overlay import lower_extended_insts
from concourse.bass_utils import run_bass_kernel_spmd

# problem constants (fixed by the harness's setup_inputs)
N_NODES = 20000
N_EDGES = 320000
IN_DIM = 128
HID = 64
HEADS = 4
HC = HEADS * HID          # 256
AUG = HC + 4 * HEADS      # 272 = h | ws | 0.2ws | wd | 0.2wd
TABW = HC + 2 * HEADS     # 264 = table row payload: h | asrc | 0.2asrc
ROWE = 384                # table row stride in fp16 elems (768B)
ROW3 = 128                # layer-3 table row stride in fp16 elems (256B)
NEG = 0.2
NCORES = 8
P = 128
NBLK = 20                 # dst blocks per core (degree bands)
BPC = 5                   # bands per AG chunk
NCHUNK = NBLK // BPC      # 4
SLOTS = NBLK * P          # 2560 slots per core
TOT_SLOTS = SLOTS * NCORES
NPAD = TOT_SLOTS          # 20480 (480 pad slots)
CROWS = NCORES * BPC * P  # 5120 table rows per chunk
GMAX = 1024               # max indices per dma_gather (SWDGE ring limit)
NQUEUES = 4               # SWDGE queues (round-robin gathers across Q7 rings)
MASKV = -30000.0          # additive mask for ELL pad columns
EXP_SHIFT = -4.158883083359672   # ln(1/64): guards fp16 fold overflow

F32 = mybir.dt.float32
F16 = mybir.dt.float16
I16 = mybir.dt.int16

AF = mybir.ActivationFunctionType
OP = mybir.AluOpType


def _split_multiwaits(bir: bytes) -> bytes:
    """Walrus here allows only 1 sync wait per instruction -> hoist extras onto
    same-engine EventSemaphore waits (dedup repeated ge-waits per engine; sems
    are monotonic within the block, so a repeated >= wait is a no-op)."""
    j = orjson.loads(bir)
    ctr = 0
    for fn in j["functions"]:
        for blk in fn["blocks"]:
            out_l = []
            last_wait = {}
            for ins in blk["instructions"]:
                eng = ins.get("engine")
                si = ins.get("sync_info")
                ow = (si or {}).get("on_wait") or []
                keep = 1
                if len(ow) > keep:
                    seen = last_wait.setdefault(eng, set())
                    for w in ow[:len(ow) - keep]:
                        key = (w.get("id"), w.get("wait_mode"), w.get("wait_value"))
                        if w.get("wait_mode") == "sem-ge-imm":
                            if key in seen:
                                continue
                            seen.add(key)
                        ctr += 1
                        out_l.append({
                            "engine": eng, "ins": [], "outs": [],
                            "name": f"mwsplit-{ctr}", "opcode": "EventSemaphore",
                            "sync_info": {"on_update": [], "on_wait": [w]},
                        })
                    si["on_wait"] = ow[len(ow) - keep:]
                out_l.append(ins)
            blk["instructions"] = out_l
    return orjson.dumps(j)


class BassOneWait(bass.Bass):
    def to_json_bytes(self):
        return _split_multiwaits(super().to_json_bytes())


# ---------------------------------------------------------------- host prep

def _row_of(c, g, p):
    """Chunk-major global table row of (core c, band g, slot p)."""
    k = g // BPC
    return k * CROWS + c * (BPC * P) + (g % BPC) * P + p


def _preprocess(edge_index):
    """Degree-sorted band assignment + ELL edge layout + gather index arrays."""
    src = np.asarray(edge_index[0], dtype=np.int64)
    dst = np.asarray(edge_index[1], dtype=np.int64)
    loops = np.arange(N_NODES, dtype=np.int64)
    src = np.concatenate([src, loops])
    dst = np.concatenate([dst, loops])

    deg = np.zeros(NPAD, np.int64)
    deg[:N_NODES] = np.bincount(dst, minlength=N_NODES)

    order = np.argsort(-deg, kind="stable")          # rank -> node
    rank = np.empty(NPAD, np.int64)
    rank[order] = np.arange(NPAD)

    Tg = tuple(max(int(deg[order[g * 1024]]), 1) for g in range(NBLK))
    goff = np.concatenate([[0], np.cumsum(Tg)])
    NTT = int(goff[-1])

    g_of = rank // 1024
    w = rank % 1024
    c_of = w // P
    p_of = w % P
    grow = _row_of(c_of, g_of, p_of)                 # node -> global table row

    eord = np.argsort(dst, kind="stable")
    dsts = dst[eord]
    srcs = src[eord]
    starts = np.searchsorted(dsts, np.arange(N_NODES + 1))
    j = np.arange(len(dsts)) - starts[dsts]

    ec = c_of[dsts]
    ep = p_of[dsts]
    ecol = goff[g_of[dsts]] + j

    idx_flat = np.zeros((NCORES, NTT * P), np.int16)
    idx_flat[ec, ecol * P + ep] = grow[srcs].astype(np.int16)
    mask = np.full((NCORES, P, NTT), MASKV, np.float16)
    mask[ec, ep, ecol] = 0.0

    idx_w = np.empty((NCORES, P, NTT * 8), np.int16)
    for c in range(NCORES):
        w16 = idx_flat[c].reshape(NTT * 8, 16).T     # [16, NTT*8]
        idx_w[c] = np.tile(w16, (8, 1))

    return Tg, NTT, order, idx_w, mask


def _aug_weights(W, a_src, a_dst, heads, hid):
    """[W | ws | 0.2ws | wd | 0.2wd]; ws[:,h] = W[:, h*hid:(h+1)*hid] @ a_src[h]."""
    cin = W.shape[0]
    ws = np.zeros((cin, heads), np.float32)
    wd = np.zeros((cin, heads), np.float32)
    for h in range(heads):
        blk = W[:, h * hid:(h + 1) * hid]
        ws[:, h] = blk @ a_src[h]
        wd[:, h] = blk @ a_dst[h]
    return np.concatenate([W, ws, NEG * ws, wd, NEG * wd], axis=1).astype(np.float32)


# ---------------------------------------------------------------- device kernel

def _build(Tg):
    NTT = sum(Tg)
    TMAX = max(Tg)
    goff = [0]
    for t in Tg:
        goff.append(goff[-1] + t)

    nc = BassOneWait(num_swdge_queues=NQUEUES)
    dp = nc.declare_dram_parameter
    x_in = dp("x_in", [SLOTS, IN_DIM], F32, isOutput=False)
    idx_in = dp("idx_in", [P, NTT * 8], I16, isOutput=False)
    mask_in = dp("mask_in", [P, NTT], F16, isOutput=False)
    wa1_in = dp("wa1_in", [IN_DIM, AUG], F32, isOutput=False)
    wa2_in = dp("wa2_in", [HC, AUG], F32, isOutput=False)
    w3_in = dp("w3_in", [1, HC], F32, isOutput=False)
    c3_in = dp("c3_in", [1, 4], F32, isOutput=False)   # a_src3, a_dst3, b3, 0.2*a_src3
    b1_in = dp("b1_in", [1, HC], F32, isOutput=False)
    b2_in = dp("b2_in", [1, HC], F32, isOutput=False)
    ident_in = dp("ident_in", [P, P], F32, isOutput=False)
    out_p = dp("out_p", [P, NBLK], F32, isOutput=True)

    tab_sh = [nc.dram_tensor(f"tab_sh{l}", [SLOTS, ROWE], F16) for l in (1, 2)]
    tab_full = [nc.dram_tensor(f"tab_full{l}", [TOT_SLOTS, ROWE], F16) for l in (1, 2)]
    tab3_sh = nc.dram_tensor("tab3_sh", [SLOTS, ROW3], F16)
    tab3_full = nc.dram_tensor("tab3_full", [TOT_SLOTS, ROW3], F16)

    groups = [list(range(NCORES))]

    with tile.TileContext(nc) as tc, ExitStack() as ctx:
        consts = ctx.enter_context(tc.tile_pool(name="consts", bufs=1))
        meta = ctx.enter_context(tc.tile_pool(name="meta", bufs=1))
        state = ctx.enter_context(tc.tile_pool(name="state", bufs=1))
        gpool = ctx.enter_context(tc.tile_pool(name="gpool", bufs=2))
        sm = ctx.enter_context(tc.tile_pool(name="sm", bufs=2))
        tl = ctx.enter_context(tc.tile_pool(name="tl", bufs=2))
        psd = ctx.enter_context(tc.tile_pool(name="psd", bufs=2, space="PSUM"))
        pst = ctx.enter_context(tc.tile_pool(name="pst", bufs=2, space="PSUM"))

        nc.gpsimd.load_library(library_config.mlp)

        # ---- constants / metadata
        ident = consts.tile([P, P], F32)
        nc.sync.dma_start(out=ident, in_=ident_in[:])
        wa1 = consts.tile([P, AUG], F32)
        nc.sync.dma_start(out=wa1, in_=wa1_in[:])
        wa2 = consts.tile([P, 2, AUG], F32)
        nc.sync.dma_start(out=wa2, in_=wa2_in.rearrange("(j p) a -> p j a", p=P))

        def rep_load(name, srct, n, dt):
            t = consts.tile([P, n], dt, tag=name)
            bc = bass.AP(tensor=srct.tensor, offset=0, ap=[[0, P], [1, n]])
            nc.sync.dma_start(out=t, in_=bc)
            return t
        w3r = rep_load("w3r", w3_in[:], HC, F32)
        c3 = rep_load("c3", c3_in[:], 4, F32)
        b1r = rep_load("b1r", b1_in[:], HC, F32)
        b2r = rep_load("b2r", b2_in[:], HC, F32)

        eshift = consts.tile([P, 1], F32, tag="eshift")
        nc.vector.memset(eshift[:], EXP_SHIFT)

        idx = meta.tile([P, NTT * 8], I16)
        nc.sync.dma_start(out=idx, in_=idx_in[:])
        msk = meta.tile([P, NTT], F16)
        nc.sync.dma_start(out=msk, in_=mask_in[:])

        xin = state.tile([P, NBLK, IN_DIM], F32)
        nc.sync.dma_start(out=xin, in_=x_in.rearrange("(b p) d -> p b d", p=P))

        hprev = state.tile([P, NBLK, HC], F32)
        hprev2 = state.tile([P, NBLK, HC], F32)
        hT = state.tile([P, 2 * NBLK, P], F32)
        adl0 = state.tile([P, NBLK, 2 * HEADS], F32, tag="adl0")
        adl1 = state.tile([P, NBLK, 2 * HEADS], F32, tag="adl1")
        adls = [adl0, adl1]
        coll = state.tile([P, NBLK, TABW], F16)      # fold results (num|den)
        coll3 = state.tile([P, NBLK, 2], F16)
        h3sb = state.tile([P, NBLK, 1], F32)
        h316 = state.tile([P, NBLK, 1], F16)
        hd3 = state.tile([P, NBLK, 1], F32)          # a_dst3 * h3
        hd3l = state.tile([P, NBLK, 1], F32)         # 0.2 * a_dst3 * h3
        outsb = state.tile([P, NBLK], F32)

        def ap_of(t_slice, ap):
            return bass.AP(tensor=t_slice.tensor, offset=t_slice.offset, ap=ap)

        def transpose_into(src_view, dst_col):
            tp = pst.tile([P, P], F32, tag="tr")
            nc.tensor.transpose(out=tp, in_=src_view, identity=ident)
            nc.vector.tensor_copy(out=hT[:, dst_col, :], in_=tp)

        def dense_block(lidx, g):
            """h_aug for block g of layer lidx (0/1) -> table row + resident adl."""
            if lidx == 0:
                transpose_into(xin[:, g, :], g)
                cin_tiles = 1
            else:
                transpose_into(hprev[:, g, 0:P], 2 * g)
                transpose_into(hprev[:, g, P:HC], 2 * g + 1)
                cin_tiles = 2
            ps = psd.tile([P, AUG], F32, tag="dense")
            for jj in range(cin_tiles):
                lhsT = hT[:, cin_tiles * g + jj, :]
                rhs = wa1[:, :] if lidx == 0 else wa2[:, jj, :]
                nc.tensor.matmul(ps, lhsT, rhs,
                                 start=(jj == 0), stop=(jj == cin_tiles - 1))
            tabt = sm.tile([P, TABW], F16, tag="tabt")
            nc.vector.tensor_copy(out=tabt, in_=ps[:, 0:TABW])
            nc.sync.dma_start(
                out=tab_sh[lidx].rearrange("(g p) e -> p g e", p=P)[:, g, 0:TABW],
                in_=tabt)
            nc.vector.tensor_copy(out=adls[lidx][:, g, :], in_=ps[:, TABW:AUG])

        def ag_chunk(tin, tout, k):
            nc.gpsimd.collective_compute(
                "AllGather", OP.bypass, replica_groups=groups,
                ins=[tin[BPC * P * k: BPC * P * (k + 1), :]],
                outs=[tout[CROWS * k: CROWS * (k + 1), :]])

        nidx_regs = {}
        qctr = [0]

        def gathers(table, elem, out_view, g):
            T = Tg[g]
            c0 = 0
            while c0 < T:
                ncols = min(GMAX // P, T - c0)
                if ncols not in nidx_regs:
                    nidx_regs[ncols] = nc.gpsimd.to_reg(P * ncols)
                col = goff[g] + c0
                nc.gpsimd.dma_gather(
                    out_ap=out_view[:, c0:c0 + ncols, :],
                    in_ap=table[:],
                    idxs_ap=idx[:, 8 * col: 8 * (col + ncols)],
                    num_idxs=P * ncols, num_idxs_reg=nidx_regs[ncols],
                    elem_size=elem, queue_num=qctr[0])
                qctr[0] = (qctr[0] + 1) % NQUEUES
                c0 += ncols

        def fold_cols(t, T, w):
            n = T
            while n > 1:
                if n % 2 == 1:
                    nc.vector.tensor_tensor(
                        out=t[:, 0:1, 0:w], in0=t[:, 0:1, 0:w],
                        in1=t[:, n - 1:n, 0:w], op=OP.add)
                    n -= 1
                h = n // 2
                nc.vector.tensor_tensor(
                    out=t[:, 0:h, 0:w], in0=t[:, 0:h, 0:w],
                    in1=t[:, h:2 * h, 0:w], op=OP.add)
                n = h

        # ---------------- layer 1/2 edge phase stages
        def stage_a(lidx, g):
            T = Tg[g]
            adl = adls[lidx]
            hg = gpool.tile([P, TMAX, ROWE], F16, tag="hg")
            gathers(tab_full[lidx], ROWE, hg, g)
            e = sm.tile([P, TMAX, HEADS], F32, tag="e")
            e2 = sm.tile([P, TMAX, HEADS], F32, tag="e2")
            adl_b = ap_of(adl[:, g, 0:HEADS], [list(adl.ap[0]), [0, T], [1, HEADS]])
            adl2_b = ap_of(adl[:, g, HEADS:2 * HEADS],
                           [list(adl.ap[0]), [0, T], [1, HEADS]])
            nc.vector.tensor_tensor(out=e[:, :T, :], in0=hg[:, :T, HC:HC + HEADS],
                                    in1=adl_b, op=OP.add)
            nc.vector.tensor_tensor(out=e2[:, :T, :],
                                    in0=hg[:, :T, HC + HEADS:HC + 2 * HEADS],
                                    in1=adl2_b, op=OP.add)
            nc.vector.tensor_tensor(out=e[:, :T, :], in0=e[:, :T, :],
                                    in1=e2[:, :T, :], op=OP.max)
            msk_b = ap_of(msk[:, goff[g]:goff[g] + T],
                          [list(msk.ap[0]), [msk.ap[1][0], T], [0, HEADS]])
            nc.vector.tensor_tensor(out=e[:, :T, :], in0=e[:, :T, :],
                                    in1=msk_b, op=OP.add)
            exf = sm.tile([P, TMAX, HEADS], F16, tag="exf")
            nc.scalar.activation(out=exf[:, :T, :], in_=e[:, :T, :],
                                 func=AF.Exp, bias=eshift[:, :])
            return hg, exf

        def stage_b(g, hg, exf):
            T = Tg[g]
            exf_b = ap_of(exf[:, 0:T, :],
                          [list(exf.ap[0]), [HEADS, T], [1, HEADS], [0, HID]])
            hg4 = hg[:, 0:T, 0:HC].rearrange("p t (h c) -> p t h c", h=HEADS)
            nc.vector.tensor_tensor(out=hg4, in0=hg4, in1=exf_b, op=OP.mult)
            nc.vector.tensor_copy(out=hg[:, :T, HC:HC + HEADS], in_=exf[:, :T, :])
            fold_cols(hg, T, TABW)
            nc.vector.tensor_copy(out=coll[:, g, :], in_=hg[:, 0, 0:TABW])

        def tail_chunk(k, brow, hout):
            s = slice(BPC * k, BPC * (k + 1))
            den = tl.tile([P, BPC, HEADS], F32, tag="den")
            nc.vector.tensor_scalar_max(den, coll[:, s, HC:HC + HEADS], 1e-30)
            rec = tl.tile([P, BPC, HEADS], F32, tag="rec")
            nc.vector.reciprocal(out=rec, in_=den)
            rec_b = ap_of(rec[:, :, :],
                          [list(rec.ap[0]), [HEADS, BPC], [1, HEADS], [0, HID]])
            hn = tl.tile([P, BPC, HC], F32, tag="hn")
            nc.vector.tensor_tensor(
                out=hn.rearrange("p b (h c) -> p b h c", h=HEADS),
                in0=coll[:, s, 0:HC].rearrange("p b (h c) -> p b h c", h=HEADS),
                in1=rec_b, op=OP.mult)
            brow_b = ap_of(brow[:, :], [list(brow.ap[0]), [0, BPC], [1, HC]])
            nc.vector.tensor_tensor(out=hn, in0=hn, in1=brow_b, op=OP.add)
            r = tl.tile([P, BPC, HC], F32, tag="r")
            nc.vector.tensor_scalar_min(r, hn, 0.0)
            nc.scalar.activation(out=r, in_=r, func=AF.Exp)
            nc.vector.tensor_scalar_max(hn, hn, 0.0)
            nc.vector.tensor_tensor(out=hn, in0=hn, in1=r, op=OP.add)
            nc.vector.tensor_scalar_add(hout[:, s, :], hn, -1.0)

        def edge_layer(lidx, brow, hout, post_chunk):
            pend = None
            for g in range(NBLK):
                cur = stage_a(lidx, g)
                if pend is not None:
                    gp, hgp, exfp = pend
                    stage_b(gp, hgp, exfp)
                    if gp % BPC == BPC - 1:
                        k = gp // BPC
                        tail_chunk(k, brow, hout)
                        post_chunk(k)
                pend = (g, cur[0], cur[1])
            gp, hgp, exfp = pend
            stage_b(gp, hgp, exfp)
            tail_chunk(NCHUNK - 1, brow, hout)
            post_chunk(NCHUNK - 1)

        # ================= layer 1 dense + chunked AG
        for k in range(NCHUNK):
            for g in range(BPC * k, BPC * (k + 1)):
                dense_block(0, g)
            ag_chunk(tab_sh[0], tab_full[0], k)

        # ================= layer 1 edge (+ layer 2 dense/AG interleaved)
        def post1(k):
            for g in range(BPC * k, BPC * (k + 1)):
                dense_block(1, g)
            ag_chunk(tab_sh[1], tab_full[1], k)
        edge_layer(0, b1r, hprev, post1)

        # ================= layer 2 edge (+ layer 3 dense/AG interleaved)
        def post2(k):
            s = slice(BPC * k, BPC * (k + 1))
            for g in range(BPC * k, BPC * (k + 1)):
                tmp = sm.tile([P, HC], F32, tag="l3tmp")
                nc.vector.tensor_tensor(out=tmp, in0=hprev2[:, g, :], in1=w3r,
                                        op=OP.mult)
                nc.vector.tensor_reduce(out=h3sb[:, g, :], in_=tmp,
                                        axis=mybir.AxisListType.X, op=OP.add)
            nc.vector.tensor_copy(out=h316[:, s, :], in_=h3sb[:, s, :])
            ad3_b = ap_of(c3[:, 1:2], [list(c3.ap[0]), [0, BPC], [0, 1]])
            nc.vector.tensor_tensor(out=hd3[:, s, :], in0=h3sb[:, s, :],
                                    in1=ad3_b, op=OP.mult)
            nc.vector.tensor_scalar_mul(hd3l[:, s, :], hd3[:, s, :], NEG)
            nc.sync.dma_start(
                out=tab3_sh.rearrange("(g p) e -> p g e", p=P)[:, s, 0:1],
                in_=h316[:, s, :])
            ag_chunk(tab3_sh, tab3_full, k)
        edge_layer(1, b2r, hprev2, post2)

        # ================= layer 3 edge phase (pipelined A/B, batched tail)
        def stage_a3(g):
            T = Tg[g]
            hg = gpool.tile([P, TMAX, ROWE], F16, tag="hg")
            g3 = hg.rearrange("p t (x e) -> p (t x) e", x=ROWE // ROW3)
            gathers(tab3_full, ROW3, g3, g)
            e3 = sm.tile([P, TMAX, 1], F32, tag="e3")
            e3l = sm.tile([P, TMAX, 1], F32, tag="e3l")
            as3_b = ap_of(c3[:, 0:1], [list(c3.ap[0]), [0, T], [0, 1]])
            as3l_b = ap_of(c3[:, 3:4], [list(c3.ap[0]), [0, T], [0, 1]])
            nc.vector.tensor_tensor(out=e3[:, :T, :], in0=g3[:, :T, 0:1],
                                    in1=as3_b, op=OP.mult)
            nc.vector.tensor_tensor(out=e3l[:, :T, :], in0=g3[:, :T, 0:1],
                                    in1=as3l_b, op=OP.mult)
            hd3_b = ap_of(hd3[:, g, :], [list(hd3.ap[0]), [0, T], [1, 1]])
            hd3l_b = ap_of(hd3l[:, g, :], [list(hd3l.ap[0]), [0, T], [1, 1]])
            nc.vector.tensor_tensor(out=e3[:, :T, :], in0=e3[:, :T, :],
                                    in1=hd3_b, op=OP.add)
            nc.vector.tensor_tensor(out=e3l[:, :T, :], in0=e3l[:, :T, :],
                                    in1=hd3l_b, op=OP.add)
            nc.vector.tensor_tensor(out=e3[:, :T, :], in0=e3[:, :T, :],
                                    in1=e3l[:, :T, :], op=OP.max)
            msk_b = ap_of(msk[:, goff[g]:goff[g] + T],
                          [list(msk.ap[0]), [msk.ap[1][0], T], [0, 1]])
            nc.vector.tensor_tensor(out=e3[:, :T, :], in0=e3[:, :T, :],
                                    in1=msk_b, op=OP.add)
            ex3 = sm.tile([P, TMAX, 1], F16, tag="ex3")
            nc.scalar.activation(out=ex3[:, :T, :], in_=e3[:, :T, :],
                                 func=AF.Exp, bias=eshift[:, :])
            return hg, g3, ex3

        def stage_b3(g, g3, ex3):
            T = Tg[g]
            nc.vector.tensor_tensor(out=g3[:, :T, 0:1], in0=g3[:, :T, 0:1],
                                    in1=ex3[:, :T, :], op=OP.mult)
            nc.vector.tensor_copy(out=g3[:, :T, 1:2], in_=ex3[:, :T, :])
            fold_cols(g3, T, 2)
            nc.vector.tensor_copy(out=coll3[:, g, :], in_=g3[:, 0, 0:2])

        pend = None
        for g in range(NBLK):
            cur = stage_a3(g)
            if pend is not None:
                stage_b3(pend[0], pend[1], pend[2])
            pend = (g, cur[1], cur[2])
        stage_b3(pend[0], pend[1], pend[2])

        den3 = tl.tile([P, NBLK, 1], F32, tag="den3")
        nc.vector.tensor_scalar_max(den3, coll3[:, :, 1:2], 1e-30)
        rec3 = tl.tile([P, NBLK, 1], F32, tag="rec3")
        nc.vector.reciprocal(out=rec3, in_=den3)
        outsb3 = ap_of(outsb[:, :], [list(outsb.ap[0]), [1, NBLK], [1, 1]])
        nc.vector.tensor_tensor(out=outsb3, in0=coll3[:, :, 0:1], in1=rec3,
                                op=OP.mult)
        b3_b = ap_of(c3[:, 2:3], [list(c3.ap[0]), [0, NBLK]])
        nc.vector.tensor_tensor(out=outsb, in0=outsb, in1=b3_b, op=OP.add)
        nc.sync.dma_start(out=out_p[:], in_=outsb)

    lower_extended_insts(nc)
    return nc


_CACHE = {}


def kernel(x, edge_index, W1, a_src1, a_dst1, b1, W2, a_src2, a_dst2, b2,
           W3, a_src3, a_dst3, b3):
    Tg, NTT, order, idx_w, mask = _preprocess(np.asarray(edge_index))

    wa1 = _aug_weights(np.asarray(W1, np.float32), np.asarray(a_src1, np.float32),
                       np.asarray(a_dst1, np.float32), HEADS, HID)
    wa2 = _aug_weights(np.asarray(W2, np.float32), np.asarray(a_src2, np.float32),
                       np.asarray(a_dst2, np.float32), HEADS, HID)
    w3 = np.asarray(W3, np.float32).reshape(1, HC)
    a_s3 = float(np.asarray(a_src3).reshape(-1)[0])
    c3 = np.array([[a_s3,
                    float(np.asarray(a_dst3).reshape(-1)[0]),
                    float(np.asarray(b3).reshape(-1)[0]), NEG * a_s3]], np.float32)
    b1r = np.asarray(b1, np.float32).reshape(1, HC)
    b2r = np.asarray(b2, np.float32).reshape(1, HC)

    x = np.asarray(x, np.float32)
    in_maps = []
    for c in range(NCORES):
        r = (np.arange(NBLK)[:, None] * 1024 + c * P + np.arange(P)[None, :])
        nodes = order[r.reshape(-1)]                 # [2560] slot-major
        xs = np.zeros((SLOTS, IN_DIM), np.float32)
        valid = nodes < N_NODES
        xs[valid] = x[nodes[valid]]
        in_maps.append({
            "x_in": xs,
            "idx_in": idx_w[c], "mask_in": mask[c],
            "wa1_in": wa1, "wa2_in": wa2, "w3_in": w3, "c3_in": c3,
            "b1_in": b1r, "b2_in": b2r,
            "ident_in": np.eye(P, dtype=np.float32),
        })

    if Tg not in _CACHE:
        _CACHE[Tg] = _build(Tg)
    nc = _CACHE[Tg]
    res = run_bass_kernel_spmd(nc, in_maps, list(range(NCORES)))

    out = np.empty(N_NODES, np.float32)
    for c in range(NCORES):
        o = np.asarray(res.results[c]["out_p"])      # [P, NBLK]
        r = (np.arange(NBLK)[:, None] * 1024 + c * P + np.arange(P)[None, :])
        nodes = order[r.reshape(-1)]
        vals = o.T.reshape(-1)                       # slot-major: g*P + p
        valid = nodes < N_NODES
        out[nodes[valid]] = vals[valid]
    return out


# revision 32
# speedup vs baseline: 1.8114x; 1.0242x over previous
"""3-layer GAT (PyG GATConv semantics) on 8 Trainium2 NeuronCores — v4.

Strategy (dst-sharded, CSR-ELL, batched dma_gather, chunked collectives):
- Nodes sorted by in-degree, grouped into 20 degree-bands of 1024; band g gives
  one 128-node block to each of the 8 cores with a shared column count
  Tg[g] = max degree in the band. Edge layout per block is ELL: partition =
  dst slot, free column j = j-th incoming edge (~8% padding).
- Table rows are numbered CHUNK-MAJOR (4 chunks of 5 bands) so each chunk's
  AllGather reads/writes contiguous rows; chunk AGs of layer l+1 fire while
  layer l's edge phase is still running (dense of l+1 is interleaved per
  chunk into l's edge phase).
- Dense: h_aug = h @ [W | ws | 0.2ws | wd | 0.2wd] per block ([128, 272] PSUM);
  cols 0:264 ([h | asrc | 0.2asrc]) go fp16 into the 768B-stride table row,
  cols 264:272 (adst, 0.2adst) stay SBUF-resident.
- Edge phase per block: batched InstDMAGatherAnt (mlp GPSIMD library, int16
  indices, <=1024 idxs/instruction, round-robin over 4 SWDGE queues) pulls rows
  into ELL position. leaky = max(asrc+adst, 0.2asrc+0.2adst) (prescaled, no
  tensor_scalar), + pad mask, exp on the scalar engine with a -ln(64) bias
  (fp16 fold-overflow guard; cancels in the softmax ratio). Messages multiply
  in place; aggregation + denominator = free-axis halving-tree fold. Stages are
  software-pipelined across blocks (A(g) issued before B(g-1)) and the
  normalize+bias+ELU tail is batched per 5-block chunk.
- Layer 3 (heads=1, C=1): same scheme over a 256B-row scalar table; h3[dst]
  comes from the resident dense output.

The walrus in this toolchain accepts only ONE sync wait per instruction;
BassOneWait splits Tile-generated multi-waits into single-wait EventSemaphore
ops at serialization.
"""
import numpy as np
from contextlib import ExitStack

import orjson
import concourse.bass as bass
import concourse.tile as tile
from concourse import mybir, library_config
from concourse.library_overlay import lower_extended_insts
from concourse.bass_utils import run_bass_kernel_spmd

# problem constants (fixed by the harness's setup_inputs)
N_NODES = 20000
N_EDGES = 320000
IN_DIM = 128
HID = 64
HEADS = 4
HC = HEADS * HID          # 256
AUG = HC + 4 * HEADS      # 272 = h | ws | 0.2ws | wd | 0.2wd
TABW = HC + 2 * HEADS     # 264 = table row payload: h | asrc | 0.2asrc
ROWE = 384                # table row stride in fp16 elems (768B)
ROW3 = 128                # layer-3 table row stride in fp16 elems (256B)
NEG = 0.2
NCORES = 8
P = 128
NBLK = 20                 # dst blocks per core (degree bands)
BPC = 5                   # bands per AG chunk
NCHUNK = NBLK // BPC      # 4
SLOTS = NBLK * P          # 2560 slots per core
TOT_SLOTS = SLOTS * NCORES
NPAD = TOT_SLOTS          # 20480 (480 pad slots)
CROWS = NCORES * BPC * P  # 5120 table rows per chunk
CSTRIDE = CROWS           # no sacrificial row: Local AG, offset-0 outputs
TOT_TAB = NCHUNK * CSTRIDE
GMAX = 1024               # max indices per dma_gather (SWDGE ring limit)
NQUEUES = 4               # SWDGE queues (round-robin gathers across Q7 rings)
UMAX = 24                 # max ELL columns per gather unit (bounds hg tile)
MASKV = -30000.0          # additive mask for ELL pad columns
EXP_SHIFT = -4.158883083359672   # ln(1/64): guards fp16 fold overflow

F32 = mybir.dt.float32
F16 = mybir.dt.float16
I16 = mybir.dt.int16

AF = mybir.ActivationFunctionType
OP = mybir.AluOpType


def _split_multiwaits(bir: bytes) -> bytes:
    """Walrus here allows only 1 sync wait per instruction -> hoist extras onto
    same-engine EventSemaphore waits (dedup repeated ge-waits per engine; sems
    are monotonic within the block, so a repeated >= wait is a no-op)."""
    j = orjson.loads(bir)
    ctr = 0
    for fn in j["functions"]:
        for blk in fn["blocks"]:
            out_l = []
            last_wait = {}
            for ins in blk["instructions"]:
                eng = ins.get("engine")
                si = ins.get("sync_info")
                ow = (si or {}).get("on_wait") or []
                keep = 1
                if len(ow) > keep:
                    seen = last_wait.setdefault(eng, set())
                    for w in ow[:len(ow) - keep]:
                        key = (w.get("id"), w.get("wait_mode"), w.get("wait_value"))
                        if w.get("wait_mode") == "sem-ge-imm":
                            if key in seen:
                                continue
                            seen.add(key)
                        ctr += 1
                        out_l.append({
                            "engine": eng, "ins": [], "outs": [],
                            "name": f"mwsplit-{ctr}", "opcode": "EventSemaphore",
                            "sync_info": {"on_update": [], "on_wait": [w]},
                        })
                    si["on_wait"] = ow[len(ow) - keep:]
                out_l.append(ins)
            blk["instructions"] = out_l
    return orjson.dumps(j)


class BassOneWait(bass.Bass):
    def to_json_bytes(self):
        return _split_multiwaits(super().to_json_bytes())


# ---------------------------------------------------------------- host prep

def _row_of(c, g, p):
    """Chunk-major global table row of (core c, band g, slot p)."""
    k = g // BPC
    return k * CSTRIDE + c * (BPC * P) + (g % BPC) * P + p


def _preprocess(edge_index):
    """Degree-sorted band assignment + ELL edge layout + gather index arrays."""
    src = np.asarray(edge_index[0], dtype=np.int64)
    dst = np.asarray(edge_index[1], dtype=np.int64)
    loops = np.arange(N_NODES, dtype=np.int64)
    src = np.concatenate([src, loops])
    dst = np.concatenate([dst, loops])

    deg = np.zeros(NPAD, np.int64)
    deg[:N_NODES] = np.bincount(dst, minlength=N_NODES)

    order = np.argsort(-deg, kind="stable")          # rank -> node
    rank = np.empty(NPAD, np.int64)
    rank[order] = np.arange(NPAD)

    Tg = tuple(max(int(deg[order[g * 1024]]), 1) for g in range(NBLK))
    goff = np.concatenate([[0], np.cumsum(Tg)])
    NTT = int(goff[-1])

    g_of = rank // 1024
    w = rank % 1024
    c_of = w // P
    p_of = w % P
    grow = _row_of(c_of, g_of, p_of)                 # node -> global table row

    eord = np.argsort(dst, kind="stable")
    dsts = dst[eord]
    srcs = src[eord]
    starts = np.searchsorted(dsts, np.arange(N_NODES + 1))
    j = np.arange(len(dsts)) - starts[dsts]

    ec = c_of[dsts]
    ep = p_of[dsts]
    ecol = goff[g_of[dsts]] + j

    idx_flat = np.zeros((NCORES, NTT * P), np.int16)
    idx_flat[ec, ecol * P + ep] = grow[srcs].astype(np.int16)
    mask = np.full((NCORES, P, NTT), MASKV, np.float16)
    mask[ec, ep, ecol] = 0.0

    idx_w = np.empty((NCORES, P, NTT * 8), np.int16)
    for c in range(NCORES):
        w16 = idx_flat[c].reshape(NTT * 8, 16).T     # [16, NTT*8]
        idx_w[c] = np.tile(w16, (8, 1))

    return Tg, NTT, order, idx_w, mask


def _aug_weights(W, a_src, a_dst, heads, hid):
    """[W | ws | 0.2ws | wd | 0.2wd]; ws[:,h] = W[:, h*hid:(h+1)*hid] @ a_src[h]."""
    cin = W.shape[0]
    ws = np.zeros((cin, heads), np.float32)
    wd = np.zeros((cin, heads), np.float32)
    for h in range(heads):
        blk = W[:, h * hid:(h + 1) * hid]
        ws[:, h] = blk @ a_src[h]
        wd[:, h] = blk @ a_dst[h]
    return np.concatenate([W, ws, NEG * ws, wd, NEG * wd], axis=1).astype(np.float32)


# ---------------------------------------------------------------- device kernel

def _build(Tg):
    NTT = sum(Tg)
    TMAX = max(Tg)
    goff = [0]
    for t in Tg:
        goff.append(goff[-1] + t)

    nc = BassOneWait(num_swdge_queues=NQUEUES)
    dp = nc.declare_dram_parameter
    x_in = dp("x_in", [SLOTS, IN_DIM], F32, isOutput=False)
    idx_in = dp("idx_in", [P, NTT * 8], I16, isOutput=False)
    mask_in = dp("mask_in", [P, NTT], F16, isOutput=False)
    wa1_in = dp("wa1_in", [IN_DIM, AUG], F32, isOutput=False)
    wa2_in = dp("wa2_in", [HC, AUG], F32, isOutput=False)
    w3_in = dp("w3_in", [1, HC], F32, isOutput=False)
    c3_in = dp("c3_in", [1, 4], F32, isOutput=False)   # a_src3, a_dst3, b3, 0.2*a_src3
    b1_in = dp("b1_in", [1, HC], F32, isOutput=False)
    b2_in = dp("b2_in", [1, HC], F32, isOutput=False)
    ident_in = dp("ident_in", [P, P], F32, isOutput=False)
    out_p = dp("out_p", [P, NBLK], F32, isOutput=True)

    tab_sh = [nc.dram_tensor(f"tab_sh{l}", [SLOTS, ROWE], F16) for l in (1, 2)]
    tab_full = [nc.dram_tensor(f"tab_full{l}", [TOT_TAB, ROWE], F16)
                for l in (1, 2)]
    tab3_sh = nc.dram_tensor("tab3_sh", [SLOTS, ROW3], F16)
    tab3_full = nc.dram_tensor("tab3_full", [TOT_TAB, ROW3], F16)

    groups = [list(range(NCORES))]

    with tile.TileContext(nc) as tc, ExitStack() as ctx:
        consts = ctx.enter_context(tc.tile_pool(name="consts", bufs=1))
        meta = ctx.enter_context(tc.tile_pool(name="meta", bufs=1))
        state = ctx.enter_context(tc.tile_pool(name="state", bufs=1))
        gpool = ctx.enter_context(tc.tile_pool(name="gpool", bufs=3))
        sm = ctx.enter_context(tc.tile_pool(name="sm", bufs=2))
        tl = ctx.enter_context(tc.tile_pool(name="tl", bufs=2))
        psd = ctx.enter_context(tc.tile_pool(name="psd", bufs=2, space="PSUM"))
        pst = ctx.enter_context(tc.tile_pool(name="pst", bufs=2, space="PSUM"))

        nc.gpsimd.load_library(library_config.mlp)

        # ---- constants / metadata
        ident = consts.tile([P, P], F32)
        nc.sync.dma_start(out=ident, in_=ident_in[:])
        wa1 = consts.tile([P, AUG], F32)
        nc.sync.dma_start(out=wa1, in_=wa1_in[:])
        wa2 = consts.tile([P, 2, AUG], F32)
        nc.sync.dma_start(out=wa2, in_=wa2_in.rearrange("(j p) a -> p j a", p=P))

        def rep_load(name, srct, n, dt):
            t = consts.tile([P, n], dt, tag=name)
            bc = bass.AP(tensor=srct.tensor, offset=0, ap=[[0, P], [1, n]])
            nc.sync.dma_start(out=t, in_=bc)
            return t
        w3r = rep_load("w3r", w3_in[:], HC, F32)
        c3 = rep_load("c3", c3_in[:], 4, F32)
        b1r = rep_load("b1r", b1_in[:], HC, F32)
        b2r = rep_load("b2r", b2_in[:], HC, F32)

        eshift = consts.tile([P, 1], F32, tag="eshift")
        nc.vector.memset(eshift[:], EXP_SHIFT)

        idx = meta.tile([P, NTT * 8], I16)
        nc.sync.dma_start(out=idx, in_=idx_in[:])
        msk = meta.tile([P, NTT], F16)
        nc.sync.dma_start(out=msk, in_=mask_in[:])

        xin = state.tile([P, NBLK, IN_DIM], F32)
        nc.sync.dma_start(out=xin, in_=x_in.rearrange("(b p) d -> p b d", p=P))

        hprev = state.tile([P, NBLK, HC], F32)
        hprev2 = state.tile([P, NBLK, HC], F32)
        hT = state.tile([P, 2 * NBLK, P], F32)
        adl0 = state.tile([P, NBLK, 2 * HEADS], F32, tag="adl0")
        adl1 = state.tile([P, NBLK, 2 * HEADS], F32, tag="adl1")
        adls = [adl0, adl1]
        coll = state.tile([P, NBLK, TABW], F16)      # fold results (num|den)
        coll3 = state.tile([P, NBLK, 2], F16)
        h3sb = state.tile([P, NBLK, 1], F32)
        h316 = state.tile([P, NBLK, 1], F16)
        hd3 = state.tile([P, NBLK, 1], F32)          # a_dst3 * h3
        hd3l = state.tile([P, NBLK, 1], F32)         # 0.2 * a_dst3 * h3
        outsb = state.tile([P, NBLK], F32)

        def ap_of(t_slice, ap):
            return bass.AP(tensor=t_slice.tensor, offset=t_slice.offset, ap=ap)

        def transpose_into(src_view, dst_col):
            tp = pst.tile([P, P], F32, tag="tr")
            nc.tensor.transpose(out=tp, in_=src_view, identity=ident)
            nc.vector.tensor_copy(out=hT[:, dst_col, :], in_=tp)

        def dense_block(lidx, g):
            """h_aug for block g of layer lidx (0/1) -> table row + resident adl."""
            if lidx == 0:
                transpose_into(xin[:, g, :], g)
                cin_tiles = 1
            else:
                transpose_into(hprev[:, g, 0:P], 2 * g)
                transpose_into(hprev[:, g, P:HC], 2 * g + 1)
                cin_tiles = 2
            ps = psd.tile([P, AUG], F32, tag="dense")
            for jj in range(cin_tiles):
                lhsT = hT[:, cin_tiles * g + jj, :]
                rhs = wa1[:, :] if lidx == 0 else wa2[:, jj, :]
                nc.tensor.matmul(ps, lhsT, rhs,
                                 start=(jj == 0), stop=(jj == cin_tiles - 1))
            tabt = sm.tile([P, TABW], F16, tag="tabt")
            nc.vector.tensor_copy(out=tabt, in_=ps[:, 0:TABW])
            nc.sync.dma_start(
                out=tab_sh[lidx].rearrange("(g p) e -> p g e", p=P)[:, g, 0:TABW],
                in_=tabt)
            nc.vector.tensor_copy(out=adls[lidx][:, g, :], in_=ps[:, TABW:AUG])

        def ag_chunk(tin, tout, k):
            cc = nc.gpsimd.collective_compute(
                "AllGather", OP.bypass, replica_groups=groups,
                ins=[tin[BPC * P * k: BPC * P * (k + 1), :]],
                outs=[tout[CSTRIDE * k: CSTRIDE * k + CROWS, :]])
            return cc

        nidx_regs = {}
        qctr = [0]

        def gathers(table, elem, out_view, g, u0, u1):
            c0 = u0
            while c0 < u1:
                ncols = min(GMAX // P, u1 - c0)
                if ncols not in nidx_regs:
                    nidx_regs[ncols] = nc.gpsimd.to_reg(P * ncols)
                col = goff[g] + c0
                nc.gpsimd.dma_gather(
                    out_ap=out_view[:, c0 - u0:c0 - u0 + ncols, :],
                    in_ap=table[:],
                    idxs_ap=idx[:, 8 * col: 8 * (col + ncols)],
                    num_idxs=P * ncols, num_idxs_reg=nidx_regs[ncols],
                    elem_size=elem, queue_num=qctr[0])
                qctr[0] = (qctr[0] + 1) % NQUEUES
                c0 += ncols

        # split bands into gather units of <= UMAX columns
        units = []                         # (g, u0, u1, last_of_band)
        for g in range(NBLK):
            T = Tg[g]
            nu = -(-T // UMAX)
            step = -(-T // nu)
            c0 = 0
            while c0 < T:
                c1 = min(c0 + step, T)
                units.append((g, c0, c1, c1 == T))
                c0 = c1

        def fold_cols(t, T, w):
            n = T
            while n > 1:
                if n % 2 == 1:
                    nc.vector.tensor_tensor(
                        out=t[:, 0:1, 0:w], in0=t[:, 0:1, 0:w],
                        in1=t[:, n - 1:n, 0:w], op=OP.add)
                    n -= 1
                h = n // 2
                nc.vector.tensor_tensor(
                    out=t[:, 0:h, 0:w], in0=t[:, 0:h, 0:w],
                    in1=t[:, h:2 * h, 0:w], op=OP.add)
                n = h

        # ---------------- layer 1/2 edge phase stages (per unit)
        def stage_a(lidx, g, u0, u1):
            T = u1 - u0
            adl = adls[lidx]
            hg = gpool.tile([P, UMAX, ROWE], F16, tag="hg")
            gathers(tab_full[lidx], ROWE, hg, g, u0, u1)
            e = sm.tile([P, UMAX, HEADS], F32, tag="e")
            e2 = sm.tile([P, UMAX, HEADS], F32, tag="e2")
            adl_b = ap_of(adl[:, g, 0:HEADS], [list(adl.ap[0]), [0, T], [1, HEADS]])
            adl2_b = ap_of(adl[:, g, HEADS:2 * HEADS],
                           [list(adl.ap[0]), [0, T], [1, HEADS]])
            nc.vector.tensor_tensor(out=e[:, :T, :], in0=hg[:, :T, HC:HC + HEADS],
                                    in1=adl_b, op=OP.add)
            nc.vector.tensor_tensor(out=e2[:, :T, :],
                                    in0=hg[:, :T, HC + HEADS:HC + 2 * HEADS],
                                    in1=adl2_b, op=OP.add)
            nc.vector.tensor_tensor(out=e[:, :T, :], in0=e[:, :T, :],
                                    in1=e2[:, :T, :], op=OP.max)
            msk_b = ap_of(msk[:, goff[g] + u0:goff[g] + u1],
                          [list(msk.ap[0]), [msk.ap[1][0], T], [0, HEADS]])
            nc.vector.tensor_tensor(out=e[:, :T, :], in0=e[:, :T, :],
                                    in1=msk_b, op=OP.add)
            exf = sm.tile([P, UMAX, HEADS], F16, tag="exf")
            nc.scalar.activation(out=exf[:, :T, :], in_=e[:, :T, :],
                                 func=AF.Exp, bias=eshift[:, :])
            return hg, exf

        def stage_b(g, u0, u1, hg, exf):
            T = u1 - u0
            exf_b = ap_of(exf[:, 0:T, :],
                          [list(exf.ap[0]), [HEADS, T], [1, HEADS], [0, HID]])
            hg4 = hg[:, 0:T, 0:HC].rearrange("p t (h c) -> p t h c", h=HEADS)
            nc.vector.tensor_tensor(out=hg4, in0=hg4, in1=exf_b, op=OP.mult)
            nc.vector.tensor_copy(out=hg[:, :T, HC:HC + HEADS], in_=exf[:, :T, :])
            fold_cols(hg, T, TABW)
            if u0 == 0:
                nc.vector.tensor_copy(out=coll[:, g, :], in_=hg[:, 0, 0:TABW])
            else:
                nc.vector.tensor_tensor(out=coll[:, g, :], in0=coll[:, g, :],
                                        in1=hg[:, 0, 0:TABW], op=OP.add)

        def tail_chunk(k, brow, hout):
            s = slice(BPC * k, BPC * (k + 1))
            den = tl.tile([P, BPC, HEADS], F32, tag="den")
            nc.vector.tensor_scalar_max(den, coll[:, s, HC:HC + HEADS], 1e-30)
            rec = tl.tile([P, BPC, HEADS], F32, tag="rec")
            nc.vector.reciprocal(out=rec, in_=den)
            rec_b = ap_of(rec[:, :, :],
                          [list(rec.ap[0]), [HEADS, BPC], [1, HEADS], [0, HID]])
            hn = tl.tile([P, BPC, HC], F32, tag="hn")
            nc.vector.tensor_tensor(
                out=hn.rearrange("p b (h c) -> p b h c", h=HEADS),
                in0=coll[:, s, 0:HC].rearrange("p b (h c) -> p b h c", h=HEADS),
                in1=rec_b, op=OP.mult)
            brow_b = ap_of(brow[:, :], [list(brow.ap[0]), [0, BPC], [1, HC]])
            nc.vector.tensor_tensor(out=hn, in0=hn, in1=brow_b, op=OP.add)
            r = tl.tile([P, BPC, HC], F32, tag="r")
            nc.vector.tensor_scalar_min(r, hn, 0.0)
            nc.scalar.activation(out=r, in_=r, func=AF.Exp)
            nc.vector.tensor_scalar_max(hn, hn, 0.0)
            nc.vector.tensor_tensor(out=hn, in0=hn, in1=r, op=OP.add)
            nc.vector.tensor_scalar_add(hout[:, s, :], hn, -1.0)

        def edge_layer(lidx, brow, hout, post_chunk):
            pend = None
            for u in units:
                cur = stage_a(lidx, u[0], u[1], u[2])
                if pend is not None:
                    up, hgp, exfp = pend
                    stage_b(up[0], up[1], up[2], hgp, exfp)
                    if up[3] and up[0] % BPC == BPC - 1:
                        k = up[0] // BPC
                        tail_chunk(k, brow, hout)
                        post_chunk(k)
                pend = (u, cur[0], cur[1])
            up, hgp, exfp = pend
            stage_b(up[0], up[1], up[2], hgp, exfp)
            tail_chunk(NCHUNK - 1, brow, hout)
            post_chunk(NCHUNK - 1)

        # ================= layer 1 dense + chunked AG
        for k in range(NCHUNK):
            for g in range(BPC * k, BPC * (k + 1)):
                dense_block(0, g)
            ag_chunk(tab_sh[0], tab_full[0], k)

        # ================= layer 1 edge (+ layer 2 dense/AG interleaved)
        def post1(k):
            for g in range(BPC * k, BPC * (k + 1)):
                dense_block(1, g)
            ag_chunk(tab_sh[1], tab_full[1], k)
        edge_layer(0, b1r, hprev, post1)

        # ================= layer 2 edge (+ layer 3 dense/AG interleaved)
        def post2(k):
            s = slice(BPC * k, BPC * (k + 1))
            for g in range(BPC * k, BPC * (k + 1)):
                tmp = sm.tile([P, HC], F32, tag="l3tmp")
                nc.vector.tensor_tensor(out=tmp, in0=hprev2[:, g, :], in1=w3r,
                                        op=OP.mult)
                nc.vector.tensor_reduce(out=h3sb[:, g, :], in_=tmp,
                                        axis=mybir.AxisListType.X, op=OP.add)
            nc.vector.tensor_copy(out=h316[:, s, :], in_=h3sb[:, s, :])
            ad3_b = ap_of(c3[:, 1:2], [list(c3.ap[0]), [0, BPC], [0, 1]])
            nc.vector.tensor_tensor(out=hd3[:, s, :], in0=h3sb[:, s, :],
                                    in1=ad3_b, op=OP.mult)
            nc.vector.tensor_scalar_mul(hd3l[:, s, :], hd3[:, s, :], NEG)
            nc.sync.dma_start(
                out=tab3_sh.rearrange("(g p) e -> p g e", p=P)[:, s, 0:1],
                in_=h316[:, s, :])
            ag_chunk(tab3_sh, tab3_full, k)
        edge_layer(1, b2r, hprev2, post2)

        # ================= layer 3 edge phase (pipelined A/B, batched tail)
        def stage_a3(g, u0, u1):
            T = u1 - u0
            hg = gpool.tile([P, UMAX, ROWE], F16, tag="hg")
            g3 = hg.rearrange("p t (x e) -> p (t x) e", x=ROWE // ROW3)
            gathers(tab3_full, ROW3, g3, g, u0, u1)
            e3 = sm.tile([P, UMAX, 1], F32, tag="e3")
            e3l = sm.tile([P, UMAX, 1], F32, tag="e3l")
            as3_b = ap_of(c3[:, 0:1], [list(c3.ap[0]), [0, T], [0, 1]])
            as3l_b = ap_of(c3[:, 3:4], [list(c3.ap[0]), [0, T], [0, 1]])
            nc.vector.tensor_tensor(out=e3[:, :T, :], in0=g3[:, :T, 0:1],
                                    in1=as3_b, op=OP.mult)
            nc.vector.tensor_tensor(out=e3l[:, :T, :], in0=g3[:, :T, 0:1],
                                    in1=as3l_b, op=OP.mult)
            hd3_b = ap_of(hd3[:, g, :], [list(hd3.ap[0]), [0, T], [1, 1]])
            hd3l_b = ap_of(hd3l[:, g, :], [list(hd3l.ap[0]), [0, T], [1, 1]])
            nc.vector.tensor_tensor(out=e3[:, :T, :], in0=e3[:, :T, :],
                                    in1=hd3_b, op=OP.add)
            nc.vector.tensor_tensor(out=e3l[:, :T, :], in0=e3l[:, :T, :],
                                    in1=hd3l_b, op=OP.add)
            nc.vector.tensor_tensor(out=e3[:, :T, :], in0=e3[:, :T, :],
                                    in1=e3l[:, :T, :], op=OP.max)
            msk_b = ap_of(msk[:, goff[g] + u0:goff[g] + u1],
                          [list(msk.ap[0]), [msk.ap[1][0], T], [0, 1]])
            nc.vector.tensor_tensor(out=e3[:, :T, :], in0=e3[:, :T, :],
                                    in1=msk_b, op=OP.add)
            ex3 = sm.tile([P, UMAX, 1], F16, tag="ex3")
            nc.scalar.activation(out=ex3[:, :T, :], in_=e3[:, :T, :],
                                 func=AF.Exp, bias=eshift[:, :])
            return hg, g3, ex3

        def stage_b3(g, u0, u1, g3, ex3):
            T = u1 - u0
            nc.vector.tensor_tensor(out=g3[:, :T, 0:1], in0=g3[:, :T, 0:1],
                                    in1=ex3[:, :T, :], op=OP.mult)
            nc.vector.tensor_copy(out=g3[:, :T, 1:2], in_=ex3[:, :T, :])
            fold_cols(g3, T, 2)
            if u0 == 0:
                nc.vector.tensor_copy(out=coll3[:, g, :], in_=g3[:, 0, 0:2])
            else:
                nc.vector.tensor_tensor(out=coll3[:, g, :], in0=coll3[:, g, :],
                                        in1=g3[:, 0, 0:2], op=OP.add)

        pend = None
        for u in units:
            cur = stage_a3(u[0], u[1], u[2])
            if pend is not None:
                up, g3p, ex3p = pend
                stage_b3(up[0], up[1], up[2], g3p, ex3p)
            pend = (u, cur[1], cur[2])
        up, g3p, ex3p = pend
        stage_b3(up[0], up[1], up[2], g3p, ex3p)

        den3 = tl.tile([P, NBLK, 1], F32, tag="den3")
        nc.vector.tensor_scalar_max(den3, coll3[:, :, 1:2], 1e-30)
        rec3 = tl.tile([P, NBLK, 1], F32, tag="rec3")
        nc.vector.reciprocal(out=rec3, in_=den3)
        outsb3 = ap_of(outsb[:, :], [list(outsb.ap[0]), [1, NBLK], [1, 1]])
        nc.vector.tensor_tensor(out=outsb3, in0=coll3[:, :, 0:1], in1=rec3,
                                op=OP.mult)
        b3_b = ap_of(c3[:, 2:3], [list(c3.ap[0]), [0, NBLK]])
        nc.vector.tensor_tensor(out=outsb, in0=outsb, in1=b3_b, op=OP.add)
        nc.sync.dma_start(out=out_p[:], in_=outsb)

    lower_extended_insts(nc)
    return nc


_CACHE = {}


def kernel(x, edge_index, W1, a_src1, a_dst1, b1, W2, a_src2, a_dst2, b2,
           W3, a_src3, a_dst3, b3):
    Tg, NTT, order, idx_w, mask = _preprocess(np.asarray(edge_index))

    wa1 = _aug_weights(np.asarray(W1, np.float32), np.asarray(a_src1, np.float32),
                       np.asarray(a_dst1, np.float32), HEADS, HID)
    wa2 = _aug_weights(np.asarray(W2, np.float32), np.asarray(a_src2, np.float32),
                       np.asarray(a_dst2, np.float32), HEADS, HID)
    w3 = np.asarray(W3, np.float32).reshape(1, HC)
    a_s3 = float(np.asarray(a_src3).reshape(-1)[0])
    c3 = np.array([[a_s3,
                    float(np.asarray(a_dst3).reshape(-1)[0]),
                    float(np.asarray(b3).reshape(-1)[0]), NEG * a_s3]], np.float32)
    b1r = np.asarray(b1, np.float32).reshape(1, HC)
    b2r = np.asarray(b2, np.float32).reshape(1, HC)

    x = np.asarray(x, np.float32)
    in_maps = []
    for c in range(NCORES):
        r = (np.arange(NBLK)[:, None] * 1024 + c * P + np.arange(P)[None, :])
        nodes = order[r.reshape(-1)]                 # [2560] slot-major
        xs = np.zeros((SLOTS, IN_DIM), np.float32)
        valid = nodes < N_NODES
        xs[valid] = x[nodes[valid]]
        in_maps.append({
            "x_in": xs,
            "idx_in": idx_w[c], "mask_in": mask[c],
            "wa1_in": wa1, "wa2_in": wa2, "w3_in": w3, "c3_in": c3,
            "b1_in": b1r, "b2_in": b2r,
            "ident_in": np.eye(P, dtype=np.float32),
        })

    if Tg not in _CACHE:
        _CACHE[Tg] = _build(Tg)
    nc = _CACHE[Tg]
    res = run_bass_kernel_spmd(nc, in_maps, list(range(NCORES)))

    out = np.empty(N_NODES, np.float32)
    for c in range(NCORES):
        o = np.asarray(res.results[c]["out_p"])      # [P, NBLK]
        r = (np.arange(NBLK)[:, None] * 1024 + c * P + np.arange(P)[None, :])
        nodes = order[r.reshape(-1)]
        vals = o.T.reshape(-1)                       # slot-major: g*P + p
        valid = nodes < N_NODES
        out[nodes[valid]] = vals[valid]
    return out
